# revision 1
# baseline (speedup 1.0000x reference)
"""
ContentAwareUpsampling (CARAFE-style) Trainium2 Bass kernel.

x[2,256,48,48] -> out[2,256,192,192]; 8 cores = 2 batches x 4 blocks of 12 rows.

Decomposition (validated bit-close by emulate() against the jax reference):
  branch convs (bf16 matmuls, BN+relu fused into ACT drains)
  softmax via exp (ACT) + group-sum/broadcast matmuls + reciprocal
  T = collapse(bilinear(softmax)) as 9 shifted matmuls vs constant [100,144]
  band matrices B built by GPSIMD local_scatter with constant indices
  einsum  E[c,(u,q,b)] = sum_dy xT[row a+dy].T @ B[dy]     (per row a, parity p)
  proj    F[r] = sum_u P_sub[u].T @ E_sigma[r*64:+64]      (sigma baked in xt)
  pixel-shuffle + BN + relu fused into the ACT drains; DMA out.
"""

import os
import numpy as np

# ---------------------------------------------------------------- constants
N, C, H, W = 2, 256, 48, 48
Cc, CK, UP, K5 = 64, 100, 2, 5
EPS = 1e-5
NCORES = 8
BLK = 12            # lo-res rows per core block
R_XIN = 20          # x rows for branch (a0-4 .. a0+15)
R_XT = 14           # x rows for einsum (a0-1 .. a0+12)
R_KN = 14           # kernel rows (a0-1 .. a0+12)
OH, OW = H * UP, W * UP          # 96
FH, FW = OH * UP, OW * UP        # 192

DYMAP = {0: [-1, -1, 0, 0, 1], 1: [-1, 0, 0, 1, 1]}
BLEND = {0: {-1: 0.25, 0: 0.75}, 1: {0: 0.75, 1: 0.25}}


def _f32(a):
    return np.ascontiguousarray(a, dtype=np.float32)


# ---------------------------------------------------------------- host prep
def _fold_bn(w, g, b, m, v):
    inv = g / np.sqrt(v + EPS)
    return w * inv[:, None], b - m * inv


def build_c2mats():
    """C2[(rho+1)*5 + (s+2)] [100, 144]: D2[b', (p,dy,q,u,dx)] =
    sum_{rho,s} Kn[:, j+rho+1, b'+s+2].T @ C2, with tau = s + dx."""
    cm = np.zeros((15, CK, 144), dtype=np.float32)
    for p in range(2):
        for q in range(2):
            for u in range(4):
                for ky in range(K5):
                    dy = DYMAP[p][ky]
                    for kx in range(K5):
                        dx = DYMAP[q][kx]
                        col = p * 72 + (dy + 1) * 24 + q * 12 + u * 3 + (dx + 1)
                        row = u * 25 + ky * 5 + kx
                        for rho, br in BLEND[p].items():
                            for tau, bc in BLEND[q].items():
                                s = tau - dx
                                w = (rho + 1) * 5 + (s + 2)
                                cm[w, row, col] += br * bc
    return cm


def build_scat_idx():
    """Two scatter index sets (dy-pair packed einsum):
    sidx_a [128, 48]: B1 partitions 0-49 = x-row a-1 (dy_i=0, data cols 0-23),
                      partitions 64-113 = x-row a (dy_i=1, data cols 24-47).
    sidx_b [64, 24]:  B2 (dy_i=2), data cols 48-71 of the p-block.
    """
    sidx_a = np.full((128, 48), -1, dtype=np.int16)
    sidx_b = np.full((64, 24), -1, dtype=np.int16)
    for q in range(2):
        for u in range(4):
            for dx_i in range(3):
                t = q * 12 + u * 3 + dx_i
                for bp in range(50):
                    b = bp - 1 - (dx_i - 1)
                    if 0 <= b < 48:
                        pos = u * 96 + q * 48 + b
                        sidx_a[bp, t] = pos
                        sidx_a[64 + bp, 24 + t] = pos
                        sidx_b[bp, t] = pos
    return sidx_a, sidx_b


def host_prep(inputs):
    x = _f32(inputs["x"])
    W1, bias1 = _fold_bn(_f32(inputs["compress_w"])[:, :, 0, 0],
                         _f32(inputs["g1"]), _f32(inputs["b1"]),
                         _f32(inputs["m1"]), _f32(inputs["v1"]))
    invk = _f32(inputs["gk"]) / np.sqrt(_f32(inputs["vk"]) + EPS)
    W2 = _f32(inputs["ke1_w"]) * invk[:, None, None, None]
    biask = _f32(inputs["bk"]) - _f32(inputs["mk"]) * invk
    W3 = _f32(inputs["ke2_w"])
    P, bias2 = _fold_bn(_f32(inputs["proj_w"])[:, :, 0, 0],
                        _f32(inputs["g2"]), _f32(inputs["b2"]),
                        _f32(inputs["m2"]), _f32(inputs["v2"]))
    cmats = build_c2mats()
    sidx_a, sidx_b = build_scat_idx()
    # sigma position p = r*64 + m  ->  channel c = 4m + r
    sigma = np.array([4 * (p % 64) + p // 64 for p in range(256)])
    return dict(x=x, W1=W1, bias1=bias1, W2=W2, biask=biask, W3=W3,
                P=P, bias2=bias2, cmats=cmats, sidx_a=sidx_a,
                sidx_b=sidx_b, sigma=sigma)


def core_slices(hp, core):
    n, blk = core // 4, core % 4
    a0 = blk * BLK
    x = hp["x"][n]

    xin = np.zeros((2, 128, R_XIN, 52), dtype=np.float32)
    for s in range(R_XIN):
        row = a0 - 4 + s
        if 0 <= row < H:
            xin[0, :, s, 2:50] = x[:128, row, :]
            xin[1, :, s, 2:50] = x[128:, row, :]

    xt = np.zeros((50, R_XT, 2, 128), dtype=np.float32)
    xs = x[hp["sigma"]]
    for s in range(R_XT):
        row = a0 - 1 + s
        if 0 <= row < H:
            xt[1:49, s, 0, :] = xs[:128, row, :].T
            xt[1:49, s, 1, :] = xs[128:, row, :].T

    xt2 = np.zeros((128, 12, 2, 128), dtype=np.float32)
    xt2[0:50, :, :, :] = xt[:, 0:12]
    xt2[64:114, :, :, :] = xt[:, 1:13]

    e_top = 1.0 if a0 == 0 else 0.0
    e_bot = 1.0 if a0 + BLK == H else 0.0
    mask_comp = np.array([1.0 if 0 <= a0 - 4 + s < H else 0.0
                          for s in range(R_XIN)], dtype=np.float32)
    mask_k1 = np.array([1.0 if 0 <= a0 - 2 + s < H else 0.0
                        for s in range(16)], dtype=np.float32)
    return dict(a0=a0, n=n, xin=xin, xt=xt, xt2=xt2, e_top=e_top,
                e_bot=e_bot, mask_comp=mask_comp, mask_k1=mask_k1)


# ---------------------------------------------------------------- emulation
def emulate_core(hp, cs):
    xin, xt = cs["xin"], cs["xt"]
    W1, W2, W3 = hp["W1"], hp["W2"], hp["W3"]

    xin_flat = np.concatenate([xin[0], xin[1]], axis=0)
    comp = np.zeros((Cc, R_XIN, 52), dtype=np.float32)
    comp[:, :, 2:50] = np.maximum(
        np.einsum("oc,csw->osw", W1, xin_flat[:, :, 2:50])
        + hp["bias1"][:, None, None], 0.0)
    comp *= cs["mask_comp"][None, :, None]

    k1 = np.zeros((Cc, 16, 50), dtype=np.float32)
    acc = np.zeros((Cc, 16, 48), dtype=np.float32)
    for ky in range(3):
        for kx in range(3):
            sh = comp[:, ky * 2: ky * 2 + 16, kx * 2: kx * 2 + 48]
            acc += np.einsum("oi,isw->osw", W2[:, :, ky, kx], sh)
    k1[:, :, 1:49] = np.maximum(acc + hp["biask"][:, None, None], 0.0)
    k1 *= cs["mask_k1"][None, :, None]

    kc = np.zeros((CK, R_KN, 48), dtype=np.float32)
    for ky in range(3):
        for kx in range(3):
            kc += np.einsum("oi,isw->osw", W3[:, :, ky, kx],
                            k1[:, ky: ky + 14, kx: kx + 48])

    e = np.exp(kc.reshape(4, 25, R_KN, 48))
    kn_int = (e / e.sum(axis=1, keepdims=True)).reshape(CK, R_KN, 48)

    # Kn [100, 14, 54]: col i <-> kernel pixel b = i - 3; interior i 3..50
    Kn = np.zeros((CK, R_KN, 54), dtype=np.float32)
    Kn[:, :, 3:51] = kn_int
    Kn[:, :, 2] = Kn[:, :, 3]
    Kn[:, :, 51] = Kn[:, :, 50]
    if cs["e_top"] == 1.0:
        Kn[:, 0, :] = Kn[:, 1, :]
    if cs["e_bot"] == 1.0:
        Kn[:, 13, :] = Kn[:, 12, :]

    cmats, P = hp["cmats"], hp["P"]
    scat_idx = np.full((64, 72), -1, dtype=np.int16)
    for bp in range(50):
        for dy_i in range(3):
            for q in range(2):
                for u in range(4):
                    for dx_i in range(3):
                        t = dy_i * 24 + q * 12 + u * 3 + dx_i
                        b = bp - 1 - (dx_i - 1)
                        if 0 <= b < 48:
                            scat_idx[bp, t] = dy_i * 384 + u * 96 + q * 48 + b
    out = np.zeros((C, 4 * BLK, FW), dtype=np.float32)
    for j in range(BLK):
        # D2 [50, (p, dy, q, u, dx)]
        D2 = np.zeros((50, 144), dtype=np.float32)
        for rho in (-1, 0, 1):
            for s in (-2, -1, 0, 1, 2):
                w = (rho + 1) * 5 + (s + 2)
                D2 += Kn[:, j + rho + 1, s + 2: s + 52].T @ cmats[w]
        for p in range(2):
            D = np.zeros((64, 72), dtype=np.float32)
            D[0:50] = D2[:, p * 72:(p + 1) * 72]
            B = np.zeros((64, 1152), dtype=np.float32)
            for bp in range(64):
                for t in range(72):
                    ix = scat_idx[bp, t]
                    if ix >= 0:
                        B[bp, ix] = D[bp, t]
            E = np.zeros((256, 384), dtype=np.float32)
            for ch in range(2):
                acc = np.zeros((128, 384), dtype=np.float32)
                for dy_i in range(3):
                    xsl = xt[0:50, j + dy_i, ch, :]
                    acc += xsl.T @ B[0:50, dy_i * 384:(dy_i + 1) * 384]
                E[ch * 128:(ch + 1) * 128] = acc
            for r in range(4):
                u1, u2 = r // 2, r % 2
                F = np.zeros((C, 96), dtype=np.float32)
                for u in range(4):
                    F += P[:, u * 64:(u + 1) * 64] @ \
                        E[r * 64:(r + 1) * 64, u * 96:(u + 1) * 96]
                F = np.maximum(F + hp["bias2"][:, None], 0.0)
                Y = 4 * j + 2 * p + u1
                Fq = F.reshape(C, 2, 48)
                for q in range(2):
                    out[:, Y, u2 + 2 * q::4] = Fq[:, q, :]
    return out


def emulate(inputs):
    hp = host_prep(inputs)
    out = np.zeros((N, C, FH, FW), dtype=np.float32)
    for core in range(NCORES):
        cs = core_slices(hp, core)
        n, a0 = cs["n"], cs["a0"]
        out[n, :, 4 * a0: 4 * a0 + 48, :] = emulate_core(hp, cs)
    return out


# ---------------------------------------------------------------- device
_CACHE = {}

INPUT_SPECS = [
    # branch-critical first (DMA order follows list order)
    ("xin", [2, 128, R_XIN, 52], "bf16"),
    ("w1t", [128, 2, Cc], "bf16"),
    ("bias1", [Cc, 1], "f32"),
    ("mask1", [Cc, R_XIN, 52], "bf16"),
    ("ke1t", [Cc, 9, Cc], "bf16"),
    ("biask", [Cc, 1], "f32"),
    ("mask2", [Cc, 16, 50], "bf16"),
    ("ke2t", [Cc, 9, CK], "bf16"),
    ("gden", [CK, 4], "bf16"),
    ("g2", [4, CK], "bf16"),
    ("etop", [CK, 2], "f32"),
    ("ebot", [CK, 2], "f32"),
    ("sidx_a", [128, 48], "i16"),
    ("sidx_b", [64, 24], "i16"),
    ("bias2", [128, 2, 1], "f32"),
    ("cmats", [CK, 15, 144], "bf16"),
    ("xt", [50, R_XT, 2, 128], "bf16"),
    ("xt2", [128, 12, 2, 128], "bf16"),
    ("projt", [128, 4, 2, 128], "bf16"),
]


def device_kernel(tc, outs, ins):
    """Emit the kernel. outs/ins: dicts name -> bass.AP (DRAM)."""
    from contextlib import ExitStack
    import concourse.bass as bass
    from concourse import mybir

    f32 = mybir.dt.float32
    bf16 = mybir.dt.bfloat16
    i16 = mybir.dt.int16
    AF = mybir.ActivationFunctionType
    ALU = mybir.AluOpType
    nc = tc.nc
    d_out = outs["out"]

    with ExitStack() as ctx:
        sing = ctx.enter_context(tc.tile_pool(name="sing", bufs=1))

        # ---- load inputs/constants (critical-path first, bulky ones later)
        sb = {}
        dts = {"bf16": bf16, "f32": f32, "i16": i16}
        deferred = ("xt", "xt2", "cmats", "projt")
        for name, shape, dt in INPUT_SPECS:
            t = sing.tile(shape[:] if name != "xin" else [128, R_XIN, 52],
                          dts[dt], tag=name)
            if name == "xin":
                sb["xin0"] = t
                t2 = sing.tile([128, R_XIN, 52], bf16, tag="xin1")
                sb["xin1"] = t2
                nc.sync.dma_start(t[:], ins["xin"][0])
                nc.sync.dma_start(t2[:], ins["xin"][1])
            else:
                sb[name] = t
                if name not in deferred:
                    nc.sync.dma_start(t[:], ins[name])

        # persistent activations
        dsbs = [sing.tile([128, 144], bf16, name=f"dsbr{i}") for i in range(4)]
        for t in dsbs:
            nc.vector.memset(t[:], 0.0)
        comp = sing.tile([Cc, R_XIN, 52], bf16)
        k1 = sing.tile([Cc, 16, 50], bf16)
        exps = sing.tile([CK, 672], bf16)
        recs = sing.tile([4, 672], bf16)
        kn = sing.tile([CK, R_KN, 54], bf16)
        nc.vector.memset(comp[:], 0.0)
        nc.vector.memset(k1[:], 0.0)
        nc.vector.memset(kn[:], 0.0)

        # ---- branch phase
        with tc.tile_pool(name="brp", bufs=2, space="PSUM") as brp:
            for h in range(2):
                ps = brp.tile([Cc, 10 * 48], f32, tag="psc")
                for c in range(2):
                    nc.tensor.matmul(
                        ps[:], sb["w1t"][:, c, :],
                        sb[f"xin{c}"][:, h * 10:(h + 1) * 10, 2:50],
                        start=(c == 0), stop=(c == 1))
                nc.scalar.activation(
                    comp[:, h * 10:(h + 1) * 10, 2:50],
                    ps[:].rearrange("c (r w) -> c r w", w=48),
                    AF.Relu, bias=sb["bias1"][:])
            nc.vector.tensor_mul(comp[:], comp[:], sb["mask1"][:])
            for name in deferred:
                nc.sync.dma_start(sb[name][:], ins[name])
            for h in range(2):
                ps = brp.tile([Cc, 8 * 48], f32, tag="psk1")
                for ky in range(3):
                    for kx in range(3):
                        t = ky * 3 + kx
                        nc.tensor.matmul(
                            ps[:], sb["ke1t"][:, t, :],
                            comp[:, h * 8 + ky * 2: h * 8 + ky * 2 + 8,
                                 kx * 2: kx * 2 + 48],
                            start=(t == 0), stop=(t == 8))
                nc.scalar.activation(
                    k1[:, h * 8:(h + 1) * 8, 1:49],
                    ps[:].rearrange("c (r w) -> c r w", w=48),
                    AF.Relu, bias=sb["biask"][:])
            nc.vector.tensor_mul(k1[:], k1[:], sb["mask2"][:])
            for h in range(2):
                ps = brp.tile([CK, 7 * 48], f32, tag="psk2", bufs=2)
                for ky in range(3):
                    for kx in range(3):
                        t = ky * 3 + kx
                        nc.tensor.matmul(
                            ps[:], sb["ke2t"][:, t, :],
                            k1[:, h * 7 + ky: h * 7 + ky + 7, kx: kx + 48],
                            start=(t == 0), stop=(t == 8))
                nc.scalar.activation(exps[:, h * 336:(h + 1) * 336], ps[:],
                                     AF.Exp)
            for h in range(2):
                psd = brp.tile([4, 336], f32, tag="psc")
                nc.tensor.matmul(psd[:], sb["gden"][:],
                                 exps[:, h * 336:(h + 1) * 336],
                                 start=True, stop=True)
                with nc.allow_low_precision(reason="bf16 softmax pipeline"):
                    nc.vector.reciprocal(recs[:, h * 336:(h + 1) * 336],
                                         psd[:])
            for h in range(2):
                psb = brp.tile([CK, 336], f32, tag="psc")
                nc.tensor.matmul(psb[:], sb["g2"][:],
                                 recs[:, h * 336:(h + 1) * 336],
                                 start=True, stop=True)
                nc.vector.tensor_mul(
                    kn[:, h * 7:(h + 1) * 7, 3:51],
                    exps[:, h * 336:(h + 1) * 336].rearrange(
                        "c (r w) -> c r w", w=48),
                    psb[:].rearrange("c (r w) -> c r w", w=48))
                rs = slice(h * 7, (h + 1) * 7)
                nc.vector.tensor_copy(kn[:, rs, 2:3], kn[:, rs, 3:4])
                nc.vector.tensor_copy(kn[:, rs, 51:52], kn[:, rs, 50:51])
                if h == 0:
                    nc.vector.tensor_scalar_mul(kn[:, 0, :], kn[:, 0, :],
                                                sb["etop"][:, 1:2])
                    nc.vector.scalar_tensor_tensor(
                        kn[:, 0, :], kn[:, 1, :], sb["etop"][:, 0:1],
                        kn[:, 0, :], op0=ALU.mult, op1=ALU.add)
                else:
                    nc.vector.tensor_scalar_mul(kn[:, 13, :], kn[:, 13, :],
                                                sb["ebot"][:, 1:2])
                    nc.vector.scalar_tensor_tensor(
                        kn[:, 13, :], kn[:, 12, :], sb["ebot"][:, 0:1],
                        kn[:, 13, :], op0=ALU.mult, op1=ALU.add)

        # ---- main loop
        bpool = ctx.enter_context(tc.tile_pool(name="bb", bufs=5))
        epool = ctx.enter_context(tc.tile_pool(name="esb", bufs=3))
        spool = ctx.enter_context(tc.tile_pool(name="stage", bufs=3))
        pp_t = ctx.enter_context(tc.tile_pool(name="pp_t", bufs=1, space="PSUM"))
        pp_e = ctx.enter_context(tc.tile_pool(name="pp_e", bufs=3, space="PSUM"))
        pp_f = ctx.enter_context(tc.tile_pool(name="pp_f", bufs=4, space="PSUM"))

        esb = {}
        stage = {}
        bts = {}

        def emit_tdb(j):
            # T collapse + D2 + band scatters for row j (prefetchable)
            pst = pp_t.tile([50, 144], f32, tag="t", name=f"pst{j}")
            for rho in (-1, 0, 1):
                for s in (-2, -1, 0, 1, 2):
                    w = (rho + 1) * 5 + (s + 2)
                    nc.tensor.matmul(
                        pst[:], kn[:, j + rho + 1, s + 2: s + 52],
                        sb["cmats"][:, w, :], start=(w == 0), stop=(w == 14))
            dsb = dsbs[j % 4]
            nc.vector.tensor_copy(dsb[0:50, :], pst[:])
            nc.vector.tensor_copy(dsb[64:114, :], pst[:])
            for p in range(2):
                bt = bpool.tile([128, 384], bf16, tag="b", name=f"bt{j}_{p}")
                nc.gpsimd.local_scatter(
                    bt[:], dsb[:, p * 72: p * 72 + 48],
                    sb["sidx_a"][:], channels=128,
                    num_elems=384, num_idxs=48)
                bt2 = bpool.tile([64, 384], bf16, tag="b2", name=f"bu{j}_{p}")
                nc.gpsimd.local_scatter(
                    bt2[:], dsb[0:64, p * 72 + 48: p * 72 + 72],
                    sb["sidx_b"][:], channels=64,
                    num_elems=384, num_idxs=24)
                bts[(j, p)] = (bt, bt2)

        emit_tdb(0)
        emit_tdb(1)
        for j in range(BLK):
            if j + 2 < BLK:
                emit_tdb(j + 2)
            if j % 2 == 0:
                esb = {c: epool.tile([128, 2, 4, 2, 2, 48], bf16,
                                     tag=f"e{c}", name=f"esb{c}")
                       for c in range(2)}
                stage = {m: spool.tile([128, 2, 2, 2, 48, 2, 2], f32,
                                       tag=f"s{m}", name=f"stage{m}")
                         for m in range(2)}
            for p in range(2):
                bt, bt2 = bts.pop((j, p))
                for c in range(2):
                    pse = pp_e.tile([128, 384], f32, tag="e")
                    nc.tensor.matmul(
                        pse[:], sb["xt2"][:, j, c, :], bt[:],
                        start=True, stop=False)
                    nc.tensor.matmul(
                        pse[:], sb["xt"][0:50, j + 2, c, :], bt2[0:50, :],
                        start=False, stop=True)
                    src = pse[:].rearrange("c (u q b) -> c u q b", u=4, q=2)
                    dst = esb[c][:, j % 2, :, p, :, :]
                    if (p + c) % 2 == 0:
                        nc.vector.tensor_copy(dst, src)
                    else:
                        nc.scalar.copy(dst, src)

            if j % 2 == 1:
                apair = j // 2
                for m in range(2):
                    for r in range(4):
                        u1, u2 = r // 2, r % 2
                        base = (r % 2) * 64
                        psf = pp_f.tile([128, 2 * 192], f32, tag="f")
                        for u in range(4):
                            nc.tensor.matmul(
                                psf[:], sb["projt"][base:base + 64, u, m, :],
                                esb[r // 2][base:base + 64, :, u, :, :, :],
                                start=(u == 0), stop=(u == 3))
                        for a2 in range(2):
                            src = psf[:, a2 * 192:(a2 + 1) * 192].rearrange(
                                "c (p q b) -> c p q b", p=2, q=2)
                            dst = stage[m][:, a2, :, u1, :, :, u2].rearrange(
                                "c p b q -> c p q b")
                            if (r + a2) % 2 == 0:
                                nc.scalar.activation(dst, src, AF.Relu,
                                                     bias=sb["bias2"][:, m, :])
                            else:
                                nc.vector.tensor_scalar(
                                    dst, src, sb["bias2"][:, m, :], 0.0,
                                    op0=ALU.add, op1=ALU.max)
                    for a2 in range(2):
                        for u1x in range(2):
                            base = apair * 1536 + a2 * 768 + u1x * 192
                            dsta = bass.AP(d_out[m].tensor,
                                           d_out[m].offset + base,
                                           [d_out[m].ap[0], [384, 2], [1, 192]])
                            nc.sync.dma_start(
                                dsta,
                                stage[m][:, a2, :, u1x].rearrange(
                                    "c p b q v -> c p (b q v)"))


def _build_program():
    import concourse.bacc as bacc
    import concourse.tile as tile
    from concourse import mybir

    dts = {"bf16": mybir.dt.bfloat16, "f32": mybir.dt.float32,
           "i16": mybir.dt.int16}
    nc = bacc.Bacc("TRN2", target_bir_lowering=False, debug=False,
                   num_devices=NCORES)
    ins = {}
    for name, shape, dt in INPUT_SPECS:
        ins[name] = nc.dram_tensor(name, shape, dts[dt],
                                   kind="ExternalInput").ap()
    outs = {"out": nc.dram_tensor("out", [2, 128, 48 * FW],
                                  mybir.dt.float32,
                                  kind="ExternalOutput").ap()}
    with tile.TileContext(nc) as tc:
        device_kernel(tc, outs, ins)
    nc.compile()
    return nc


def _get_program():
    if "nc" not in _CACHE:
        _CACHE["nc"] = _build_program()
    return _CACHE["nc"]


def _core_in_map(hp, core):
    import ml_dtypes
    bf16 = ml_dtypes.bfloat16
    cs = core_slices(hp, core)

    w1t = np.stack([hp["W1"][:, :128].T, hp["W1"][:, 128:].T],
                   axis=1)                            # [128, 2, 64]
    ke1t = np.zeros((Cc, 9, Cc), dtype=np.float32)
    ke2t = np.zeros((Cc, 9, CK), dtype=np.float32)
    for ky in range(3):
        for kx in range(3):
            t = ky * 3 + kx
            ke1t[:, t, :] = hp["W2"][:, :, ky, kx].T
            ke2t[:, t, :] = hp["W3"][:, :, ky, kx].T
    cmats = hp["cmats"].transpose(1, 0, 2)            # [100, 9, 144]
    projt = np.zeros((128, 4, 2, 128), dtype=np.float32)
    for u in range(4):
        for m in range(2):
            blkw = hp["P"][m * 128:(m + 1) * 128, u * 64:(u + 1) * 64].T
            projt[0:64, u, m, :] = blkw
            projt[64:128, u, m, :] = blkw
    gden = np.zeros((CK, 4), dtype=np.float32)
    for u in range(4):
        gden[u * 25:(u + 1) * 25, u] = 1.0
    g2 = np.ascontiguousarray(gden.T)
    bias2 = np.zeros((128, 2, 1), dtype=np.float32)
    bias2[:, 0, 0] = hp["bias2"][:128]
    bias2[:, 1, 0] = hp["bias2"][128:]
    etop = np.zeros((CK, 2), dtype=np.float32)
    etop[:, 0] = cs["e_top"]
    etop[:, 1] = 1.0 - cs["e_top"]
    ebot = np.zeros((CK, 2), dtype=np.float32)
    ebot[:, 0] = cs["e_bot"]
    ebot[:, 1] = 1.0 - cs["e_bot"]
    mask1 = np.broadcast_to(cs["mask_comp"][None, :, None],
                            (Cc, R_XIN, 52)).copy()
    mask2 = np.broadcast_to(cs["mask_k1"][None, :, None],
                            (Cc, 16, 50)).copy()

    return {
        "xin": cs["xin"].astype(bf16),
        "xt": cs["xt"].astype(bf16),
        "w1t": w1t.astype(bf16),
        "ke1t": ke1t.astype(bf16),
        "ke2t": ke2t.astype(bf16),
        "cmats": np.ascontiguousarray(cmats).astype(bf16),
        "projt": projt.astype(bf16),
        "gden": gden.astype(bf16),
        "g2": g2.astype(bf16),
        "sidx_a": hp["sidx_a"],
        "sidx_b": hp["sidx_b"],
        "xt2": cs["xt2"].astype(bf16),
        "mask1": mask1.astype(bf16),
        "mask2": mask2.astype(bf16),
        "bias1": np.ascontiguousarray(hp["bias1"][:, None]),
        "biask": np.ascontiguousarray(hp["biask"][:, None]),
        "bias2": bias2,
        "etop": etop,
        "ebot": ebot,
    }


def _gather(results):
    out = np.zeros((N, C, FH, FW), dtype=np.float32)
    for core in range(NCORES):
        n, blk = core // 4, core % 4
        a0 = blk * BLK
        o = results[core]["out"].reshape(2, 128, 48, FW)
        out[n, :128, 4 * a0: 4 * a0 + 48, :] = o[0]
        out[n, 128:, 4 * a0: 4 * a0 + 48, :] = o[1]
    return out


def kernel(**inputs) -> np.ndarray:
    if os.environ.get("CARAFE_EMULATE"):
        return emulate(inputs)
    from concourse.bass_utils import run_bass_kernel_spmd
    hp = host_prep(inputs)
    nc = _get_program()
    in_maps = [_core_in_map(hp, core) for core in range(NCORES)]
    res = run_bass_kernel_spmd(nc, in_maps, list(range(NCORES)),
                               trace=bool(os.environ.get("CARAFE_TRACE")))
    _CACHE["last_results"] = res
    return _gather(res.results)



# revision 20
# speedup vs baseline: 1.0460x; 1.0460x over previous
"""
ContentAwareUpsampling (CARAFE-style) Trainium2 Bass kernel.

x[2,256,48,48] -> out[2,256,192,192]; 8 cores = 2 batches x 4 blocks of 12 rows.

Decomposition (validated bit-close by emulate() against the jax reference):
  branch convs (bf16 matmuls, BN+relu fused into ACT drains)
  softmax via exp (ACT) + group-sum/broadcast matmuls + reciprocal
  T = collapse(bilinear(softmax)) as 9 shifted matmuls vs constant [100,144]
  band matrices B built by GPSIMD local_scatter with constant indices
  einsum  E[c,(u,q,b)] = sum_dy xT[row a+dy].T @ B[dy]     (per row a, parity p)
  proj    F[r] = sum_u P_sub[u].T @ E_sigma[r*64:+64]      (sigma baked in xt)
  pixel-shuffle + BN + relu fused into the ACT drains; DMA out.
"""

import os
import numpy as np

# ---------------------------------------------------------------- constants
N, C, H, W = 2, 256, 48, 48
Cc, CK, UP, K5 = 64, 100, 2, 5
EPS = 1e-5
NCORES = 8
BLK = 12            # lo-res rows per core block
R_XIN = 20          # x rows for branch (a0-4 .. a0+15)
R_XT = 14           # x rows for einsum (a0-1 .. a0+12)
R_KN = 14           # kernel rows (a0-1 .. a0+12)
OH, OW = H * UP, W * UP          # 96
FH, FW = OH * UP, OW * UP        # 192

DYMAP = {0: [-1, -1, 0, 0, 1], 1: [-1, 0, 0, 1, 1]}
BLEND = {0: {-1: 0.25, 0: 0.75}, 1: {0: 0.75, 1: 0.25}}


def _f32(a):
    return np.ascontiguousarray(a, dtype=np.float32)


# ---------------------------------------------------------------- host prep
def _fold_bn(w, g, b, m, v):
    inv = g / np.sqrt(v + EPS)
    return w * inv[:, None], b - m * inv


def build_c2mats():
    """C2[(rho+1)*5 + (s+2)] [100, 144]: D2[b', (p,dy,q,u,dx)] =
    sum_{rho,s} Kn[:, j+rho+1, b'+s+2].T @ C2, with tau = s + dx."""
    cm = np.zeros((15, CK, 144), dtype=np.float32)
    for p in range(2):
        for q in range(2):
            for u in range(4):
                for ky in range(K5):
                    dy = DYMAP[p][ky]
                    for kx in range(K5):
                        dx = DYMAP[q][kx]
                        col = p * 72 + (dy + 1) * 24 + q * 12 + u * 3 + (dx + 1)
                        row = u * 25 + ky * 5 + kx
                        for rho, br in BLEND[p].items():
                            for tau, bc in BLEND[q].items():
                                s = tau - dx
                                w = (rho + 1) * 5 + (s + 2)
                                cm[w, row, col] += br * bc
    return cm


def build_scat_idx():
    """Two scatter index sets (dy-pair packed einsum):
    sidx_a [128, 48]: B1 partitions 0-49 = x-row a-1 (dy_i=0, data cols 0-23),
                      partitions 64-113 = x-row a (dy_i=1, data cols 24-47).
    sidx_b [64, 24]:  B2 (dy_i=2), data cols 48-71 of the p-block.
    """
    sidx_a = np.full((128, 48), -1, dtype=np.int16)
    sidx_b = np.full((64, 24), -1, dtype=np.int16)
    for q in range(2):
        for u in range(4):
            for dx_i in range(3):
                t = q * 12 + u * 3 + dx_i
                for bp in range(50):
                    b = bp - 1 - (dx_i - 1)
                    if 0 <= b < 48:
                        pos = u * 96 + q * 48 + b
                        sidx_a[bp, t] = pos
                        sidx_a[64 + bp, 24 + t] = pos
                        sidx_b[bp, t] = pos
    return sidx_a, sidx_b


def host_prep(inputs):
    x = _f32(inputs["x"])
    W1, bias1 = _fold_bn(_f32(inputs["compress_w"])[:, :, 0, 0],
                         _f32(inputs["g1"]), _f32(inputs["b1"]),
                         _f32(inputs["m1"]), _f32(inputs["v1"]))
    invk = _f32(inputs["gk"]) / np.sqrt(_f32(inputs["vk"]) + EPS)
    W2 = _f32(inputs["ke1_w"]) * invk[:, None, None, None]
    biask = _f32(inputs["bk"]) - _f32(inputs["mk"]) * invk
    W3 = _f32(inputs["ke2_w"])
    P, bias2 = _fold_bn(_f32(inputs["proj_w"])[:, :, 0, 0],
                        _f32(inputs["g2"]), _f32(inputs["b2"]),
                        _f32(inputs["m2"]), _f32(inputs["v2"]))
    cmats = build_c2mats()
    sidx_a, sidx_b = build_scat_idx()
    # sigma position p = r*64 + m  ->  channel c = 4m + r
    sigma = np.array([4 * (p % 64) + p // 64 for p in range(256)])
    return dict(x=x, W1=W1, bias1=bias1, W2=W2, biask=biask, W3=W3,
                P=P, bias2=bias2, cmats=cmats, sidx_a=sidx_a,
                sidx_b=sidx_b, sigma=sigma)


def core_slices(hp, core):
    n, blk = core // 4, core % 4
    a0 = blk * BLK
    x = hp["x"][n]

    xin = np.zeros((2, 128, R_XIN, 52), dtype=np.float32)
    for s in range(R_XIN):
        row = a0 - 4 + s
        if 0 <= row < H:
            xin[0, :, s, 2:50] = x[:128, row, :]
            xin[1, :, s, 2:50] = x[128:, row, :]

    xt = np.zeros((50, R_XT, 2, 128), dtype=np.float32)
    xs = x[hp["sigma"]]
    for s in range(R_XT):
        row = a0 - 1 + s
        if 0 <= row < H:
            xt[1:49, s, 0, :] = xs[:128, row, :].T
            xt[1:49, s, 1, :] = xs[128:, row, :].T

    xt2 = np.zeros((128, 12, 2, 128), dtype=np.float32)
    xt2[0:50, :, :, :] = xt[:, 0:12]
    xt2[64:114, :, :, :] = xt[:, 1:13]

    e_top = 1.0 if a0 == 0 else 0.0
    e_bot = 1.0 if a0 + BLK == H else 0.0
    mask_comp = np.array([1.0 if 0 <= a0 - 4 + s < H else 0.0
                          for s in range(R_XIN)], dtype=np.float32)
    mask_k1 = np.array([1.0 if 0 <= a0 - 2 + s < H else 0.0
                        for s in range(16)], dtype=np.float32)
    return dict(a0=a0, n=n, xin=xin, xt=xt, xt2=xt2, e_top=e_top,
                e_bot=e_bot, mask_comp=mask_comp, mask_k1=mask_k1)


# ---------------------------------------------------------------- emulation
def emulate_core(hp, cs):
    xin, xt = cs["xin"], cs["xt"]
    W1, W2, W3 = hp["W1"], hp["W2"], hp["W3"]

    xin_flat = np.concatenate([xin[0], xin[1]], axis=0)
    comp = np.zeros((Cc, R_XIN, 52), dtype=np.float32)
    comp[:, :, 2:50] = np.maximum(
        np.einsum("oc,csw->osw", W1, xin_flat[:, :, 2:50])
        + hp["bias1"][:, None, None], 0.0)
    comp *= cs["mask_comp"][None, :, None]

    k1 = np.zeros((Cc, 16, 50), dtype=np.float32)
    acc = np.zeros((Cc, 16, 48), dtype=np.float32)
    for ky in range(3):
        for kx in range(3):
            sh = comp[:, ky * 2: ky * 2 + 16, kx * 2: kx * 2 + 48]
            acc += np.einsum("oi,isw->osw", W2[:, :, ky, kx], sh)
    k1[:, :, 1:49] = np.maximum(acc + hp["biask"][:, None, None], 0.0)
    k1 *= cs["mask_k1"][None, :, None]

    kc = np.zeros((CK, R_KN, 48), dtype=np.float32)
    for ky in range(3):
        for kx in range(3):
            kc += np.einsum("oi,isw->osw", W3[:, :, ky, kx],
                            k1[:, ky: ky + 14, kx: kx + 48])

    e = np.exp(kc.reshape(4, 25, R_KN, 48))
    kn_int = (e / e.sum(axis=1, keepdims=True)).reshape(CK, R_KN, 48)

    # Kn [100, 14, 54]: col i <-> kernel pixel b = i - 3; interior i 3..50
    Kn = np.zeros((CK, R_KN, 54), dtype=np.float32)
    Kn[:, :, 3:51] = kn_int
    Kn[:, :, 2] = Kn[:, :, 3]
    Kn[:, :, 51] = Kn[:, :, 50]
    if cs["e_top"] == 1.0:
        Kn[:, 0, :] = Kn[:, 1, :]
    if cs["e_bot"] == 1.0:
        Kn[:, 13, :] = Kn[:, 12, :]

    cmats, P = hp["cmats"], hp["P"]
    scat_idx = np.full((64, 72), -1, dtype=np.int16)
    for bp in range(50):
        for dy_i in range(3):
            for q in range(2):
                for u in range(4):
                    for dx_i in range(3):
                        t = dy_i * 24 + q * 12 + u * 3 + dx_i
                        b = bp - 1 - (dx_i - 1)
                        if 0 <= b < 48:
                            scat_idx[bp, t] = dy_i * 384 + u * 96 + q * 48 + b
    out = np.zeros((C, 4 * BLK, FW), dtype=np.float32)
    for j in range(BLK):
        # D2 [50, (p, dy, q, u, dx)]
        D2 = np.zeros((50, 144), dtype=np.float32)
        for rho in (-1, 0, 1):
            for s in (-2, -1, 0, 1, 2):
                w = (rho + 1) * 5 + (s + 2)
                D2 += Kn[:, j + rho + 1, s + 2: s + 52].T @ cmats[w]
        for p in range(2):
            D = np.zeros((64, 72), dtype=np.float32)
            D[0:50] = D2[:, p * 72:(p + 1) * 72]
            B = np.zeros((64, 1152), dtype=np.float32)
            for bp in range(64):
                for t in range(72):
                    ix = scat_idx[bp, t]
                    if ix >= 0:
                        B[bp, ix] = D[bp, t]
            E = np.zeros((256, 384), dtype=np.float32)
            for ch in range(2):
                acc = np.zeros((128, 384), dtype=np.float32)
                for dy_i in range(3):
                    xsl = xt[0:50, j + dy_i, ch, :]
                    acc += xsl.T @ B[0:50, dy_i * 384:(dy_i + 1) * 384]
                E[ch * 128:(ch + 1) * 128] = acc
            for r in range(4):
                u1, u2 = r // 2, r % 2
                F = np.zeros((C, 96), dtype=np.float32)
                for u in range(4):
                    F += P[:, u * 64:(u + 1) * 64] @ \
                        E[r * 64:(r + 1) * 64, u * 96:(u + 1) * 96]
                F = np.maximum(F + hp["bias2"][:, None], 0.0)
                Y = 4 * j + 2 * p + u1
                Fq = F.reshape(C, 2, 48)
                for q in range(2):
                    out[:, Y, u2 + 2 * q::4] = Fq[:, q, :]
    return out


def emulate(inputs):
    hp = host_prep(inputs)
    out = np.zeros((N, C, FH, FW), dtype=np.float32)
    for core in range(NCORES):
        cs = core_slices(hp, core)
        n, a0 = cs["n"], cs["a0"]
        out[n, :, 4 * a0: 4 * a0 + 48, :] = emulate_core(hp, cs)
    return out


# ---------------------------------------------------------------- device
_CACHE = {}

# packb column layout (tiny branch-critical bf16 pack, first DMA)
PB_W1T = 0          # [128, 2*64]   -> 0:128
PB_GDEN = 128       # [100, 4]      -> 128:132
PB_G2 = 132         # [4, 100]      -> 132:232
PB_M1 = 232         # [64, 20]      -> 232:252
PB_M2 = 252         # [64, 16]      -> 252:268
PB_N = 268
# packk column layout (conv weights, loads behind xin)
PK_KE1 = 0          # [64, 9*64]    -> 0:576
PK_KE2 = 576        # [64, 9*100]   -> 576:1476
PK_N = 1476

INPUT_SPECS = [
    # branch-critical first (DMA order follows list order)
    ("packb", [128, PB_N], "bf16"),
    ("xin", [2, 128, R_XIN, 52], "bf16"),
    ("packf", [128, 8], "f32"),
    ("packk", [Cc, PK_N], "bf16"),
    ("packi", [128, 72], "i16"),
    ("cmats", [CK, 15, 144], "bf16"),
    ("xt2", [128, 12, 2, 128], "bf16"),
    ("xt", [50, R_XT, 2, 128], "bf16"),
    ("projt", [128, 4, 2, 128], "bf16"),
]


def device_kernel(tc, outs, ins):
    """Emit the kernel. outs/ins: dicts name -> bass.AP (DRAM)."""
    from contextlib import ExitStack
    import concourse.bass as bass
    from concourse import mybir

    f32 = mybir.dt.float32
    bf16 = mybir.dt.bfloat16
    i16 = mybir.dt.int16
    AF = mybir.ActivationFunctionType
    ALU = mybir.AluOpType
    nc = tc.nc
    d_out = outs["out"]

    with ExitStack() as ctx:
        sing = ctx.enter_context(tc.tile_pool(name="sing", bufs=1))

        # ---- load inputs/constants (critical-path first, bulky ones later)
        sb = {}
        dts = {"bf16": bf16, "f32": f32, "i16": i16}
        deferred = ("cmats", "xt2", "xt", "projt")
        for name, shape, dt in INPUT_SPECS:
            t = sing.tile(shape[:] if name != "xin" else [128, R_XIN, 52],
                          dts[dt], tag=name)
            if name == "xin":
                sb["xin0"] = t
                t2 = sing.tile([128, R_XIN, 52], bf16, tag="xin1")
                sb["xin1"] = t2
                nc.sync.dma_start(t[:], ins["xin"][0])
                nc.sync.dma_start(t2[:], ins["xin"][1])
            else:
                sb[name] = t
                if name not in deferred:
                    nc.sync.dma_start(t[:], ins[name])

        # typed views into the packs
        pb, pf, pi = sb["packb"], sb["packf"], sb["packi"]
        pk = sb["packk"]
        sb["w1t"] = pb[:, PB_W1T:PB_W1T + 128].rearrange(
            "p (c k) -> p c k", c=2)
        sb["ke1t"] = pk[0:Cc, PK_KE1:PK_KE1 + 576].rearrange(
            "p (t k) -> p t k", t=9)
        sb["ke2t"] = pk[0:Cc, PK_KE2:PK_KE2 + 900].rearrange(
            "p (t k) -> p t k", t=9)
        sb["gden"] = pb[0:CK, PB_GDEN:PB_GDEN + 4]
        sb["g2"] = pb[0:4, PB_G2:PB_G2 + CK]
        # only the halo rows can fall outside the image: comp rows 0:4 /
        # 16:20, k1 rows 0:2 / 14:16.  Interior rows always have mask 1.
        m1a = pb[0:Cc, PB_M1:PB_M1 + 4].unsqueeze(2).broadcast_to(
            (Cc, 4, 52))
        m1b = pb[0:Cc, PB_M1 + 16:PB_M1 + 20].unsqueeze(2).broadcast_to(
            (Cc, 4, 52))
        m2a = pb[0:Cc, PB_M2:PB_M2 + 2].unsqueeze(2).broadcast_to(
            (Cc, 2, 50))
        m2b = pb[0:Cc, PB_M2 + 14:PB_M2 + 16].unsqueeze(2).broadcast_to(
            (Cc, 2, 50))
        sb["bias1"] = pf[0:Cc, 0:1]
        sb["biask"] = pf[0:Cc, 1:2]
        bias2 = [pf[:, 2:3], pf[:, 3:4]]
        sb["etop"] = pf[0:CK, 4:6]
        sb["ebot"] = pf[0:CK, 6:8]
        sb["sidx_a"] = pi[:, 0:48]
        sb["sidx_b"] = pi[0:64, 48:72]

        # persistent activations
        dsbs = [sing.tile([128, 144], bf16, name=f"dsbr{i}") for i in range(4)]
        for t in dsbs:
            nc.vector.memset(t[:], 0.0)
        comp = sing.tile([Cc, R_XIN, 52], bf16)
        k1 = sing.tile([Cc, 16, 50], bf16)
        exps = sing.tile([CK, 672], bf16)
        recs = sing.tile([4, 672], bf16)
        kn = sing.tile([CK, R_KN, 54], bf16)
        nc.vector.memset(comp[:], 0.0)
        nc.vector.memset(k1[:], 0.0)
        nc.vector.memset(kn[:], 0.0)

        # ---- branch phase
        with tc.tile_pool(name="brp", bufs=2, space="PSUM") as brp:
            for h in range(2):
                ps = brp.tile([Cc, 10 * 48], f32, tag="psc")
                for c in range(2):
                    nc.tensor.matmul(
                        ps[:], sb["w1t"][:, c, :],
                        sb[f"xin{c}"][:, h * 10:(h + 1) * 10, 2:50],
                        start=(c == 0), stop=(c == 1))
                nc.scalar.activation(
                    comp[:, h * 10:(h + 1) * 10, 2:50],
                    ps[:].rearrange("c (r w) -> c r w", w=48),
                    AF.Relu, bias=sb["bias1"][:])
                if h == 0:
                    nc.vector.tensor_mul(comp[:, 0:4, :], comp[:, 0:4, :],
                                         m1a)
                else:
                    nc.vector.tensor_mul(comp[:, 16:20, :],
                                         comp[:, 16:20, :], m1b)
            for name in deferred:
                nc.sync.dma_start(sb[name][:], ins[name])
            for h in range(2):
                ps = brp.tile([Cc, 8 * 48], f32, tag="psk1")
                for ky in range(3):
                    for kx in range(3):
                        t = ky * 3 + kx
                        nc.tensor.matmul(
                            ps[:], sb["ke1t"][:, t, :],
                            comp[:, h * 8 + ky * 2: h * 8 + ky * 2 + 8,
                                 kx * 2: kx * 2 + 48],
                            start=(t == 0), stop=(t == 8))
                nc.scalar.activation(
                    k1[:, h * 8:(h + 1) * 8, 1:49],
                    ps[:].rearrange("c (r w) -> c r w", w=48),
                    AF.Relu, bias=sb["biask"][:])
                if h == 0:
                    nc.vector.tensor_mul(k1[:, 0:2, :], k1[:, 0:2, :], m2a)
                else:
                    nc.vector.tensor_mul(k1[:, 14:16, :], k1[:, 14:16, :],
                                         m2b)
            for h in range(2):
                ps = brp.tile([CK, 7 * 48], f32, tag="psk2", bufs=2)
                for ky in range(3):
                    for kx in range(3):
                        t = ky * 3 + kx
                        nc.tensor.matmul(
                            ps[:], sb["ke2t"][:, t, :],
                            k1[:, h * 7 + ky: h * 7 + ky + 7, kx: kx + 48],
                            start=(t == 0), stop=(t == 8))
                nc.scalar.activation(exps[:, h * 336:(h + 1) * 336], ps[:],
                                     AF.Exp)
            for h in range(2):
                psd = brp.tile([4, 336], f32, tag="psc")
                nc.tensor.matmul(psd[:], sb["gden"][:],
                                 exps[:, h * 336:(h + 1) * 336],
                                 start=True, stop=True)
                with nc.allow_low_precision(reason="bf16 softmax pipeline"):
                    nc.vector.reciprocal(recs[:, h * 336:(h + 1) * 336],
                                         psd[:])
            for h in range(2):
                psb = brp.tile([CK, 336], f32, tag="psc")
                nc.tensor.matmul(psb[:], sb["g2"][:],
                                 recs[:, h * 336:(h + 1) * 336],
                                 start=True, stop=True)
                nc.vector.tensor_mul(
                    kn[:, h * 7:(h + 1) * 7, 3:51],
                    exps[:, h * 336:(h + 1) * 336].rearrange(
                        "c (r w) -> c r w", w=48),
                    psb[:].rearrange("c (r w) -> c r w", w=48))
                rs = slice(h * 7, (h + 1) * 7)
                nc.vector.tensor_copy(kn[:, rs, 2:3], kn[:, rs, 3:4])
                nc.vector.tensor_copy(kn[:, rs, 51:52], kn[:, rs, 50:51])
                if h == 0:
                    nc.vector.tensor_scalar_mul(kn[:, 0, :], kn[:, 0, :],
                                                sb["etop"][:, 1:2])
                    nc.vector.scalar_tensor_tensor(
                        kn[:, 0, :], kn[:, 1, :], sb["etop"][:, 0:1],
                        kn[:, 0, :], op0=ALU.mult, op1=ALU.add)
                else:
                    nc.vector.tensor_scalar_mul(kn[:, 13, :], kn[:, 13, :],
                                                sb["ebot"][:, 1:2])
                    nc.vector.scalar_tensor_tensor(
                        kn[:, 13, :], kn[:, 12, :], sb["ebot"][:, 0:1],
                        kn[:, 13, :], op0=ALU.mult, op1=ALU.add)

        # ---- main loop
        bpool = ctx.enter_context(tc.tile_pool(name="bb", bufs=5))
        epool = ctx.enter_context(tc.tile_pool(name="esb", bufs=3))
        spool = ctx.enter_context(tc.tile_pool(name="stage", bufs=4))
        pp_t = ctx.enter_context(tc.tile_pool(name="pp_t", bufs=1, space="PSUM"))
        pp_e = ctx.enter_context(tc.tile_pool(name="pp_e", bufs=3, space="PSUM"))
        pp_f = ctx.enter_context(tc.tile_pool(name="pp_f", bufs=4, space="PSUM"))

        esb = {}
        stage = {}
        bts = {}

        def emit_tdb(j):
            # T collapse + D2 + band scatters for row j (prefetchable)
            pst = pp_t.tile([50, 144], f32, tag="t", name=f"pst{j}")
            for rho in (-1, 0, 1):
                for s in (-2, -1, 0, 1, 2):
                    w = (rho + 1) * 5 + (s + 2)
                    nc.tensor.matmul(
                        pst[:], kn[:, j + rho + 1, s + 2: s + 52],
                        sb["cmats"][:, w, :], start=(w == 0), stop=(w == 14))
            dsb = dsbs[j % 4]
            nc.vector.tensor_copy(dsb[0:50, :], pst[:])
            nc.vector.tensor_copy(dsb[64:114, :], pst[:])
            for p in range(2):
                bt = bpool.tile([128, 384], bf16, tag="b", name=f"bt{j}_{p}")
                nc.gpsimd.local_scatter(
                    bt[:], dsb[:, p * 72: p * 72 + 48],
                    sb["sidx_a"][:], channels=128,
                    num_elems=384, num_idxs=48)
                bt2 = bpool.tile([64, 384], bf16, tag="b2", name=f"bu{j}_{p}")
                nc.gpsimd.local_scatter(
                    bt2[:], dsb[0:64, p * 72 + 48: p * 72 + 72],
                    sb["sidx_b"][:], channels=64,
                    num_elems=384, num_idxs=24)
                bts[(j, p)] = (bt, bt2)

        emit_tdb(0)
        emit_tdb(1)
        for j in range(BLK):
            if j + 2 < BLK:
                emit_tdb(j + 2)
            if j % 2 == 0:
                esb = {c: epool.tile([128, 2, 4, 2, 2, 48], bf16,
                                     tag=f"e{c}", name=f"esb{c}")
                       for c in range(2)}
                stage = {m: spool.tile([128, 8, 192], bf16,
                                       tag=f"s{m}", name=f"stage{m}")
                         for m in range(2)}
            for p in range(2):
                bt, bt2 = bts.pop((j, p))
                for c in range(2):
                    pse = pp_e.tile([128, 384], f32, tag="e")
                    nc.tensor.matmul(
                        pse[:], sb["xt2"][:, j, c, :], bt[:],
                        start=True, stop=False)
                    nc.tensor.matmul(
                        pse[:], sb["xt"][0:50, j + 2, c, :], bt2[0:50, :],
                        start=False, stop=True)
                    src = pse[:].rearrange("c (u q b) -> c u q b", u=4, q=2)
                    dst = esb[c][:, j % 2, :, p, :, :]
                    if (p + c) % 2 == 0:
                        nc.vector.tensor_copy(dst, src)
                    else:
                        nc.scalar.copy(dst, src)

            if j % 2 == 1:
                apair = j // 2
                for m in range(2):
                    for r in range(4):
                        u1, u2 = r // 2, r % 2
                        base = (r % 2) * 64
                        psf = pp_f.tile([128, 2 * 192], f32, tag="f")
                        for u in range(4):
                            nc.tensor.matmul(
                                psf[:], sb["projt"][base:base + 64, u, m, :],
                                esb[r // 2][base:base + 64, :, u, :, :, :],
                                start=(u == 0), stop=(u == 3))
                        # psf cols = (a2, p, q, b); out row Y = 4*a2+2p+u1,
                        # col X = u2 + 2q + 4b
                        src = psf[:].rearrange(
                            "c (a p q b) -> c a p q b", a=2, p=2, q=2)
                        dst = stage[m][:].rearrange(
                            "c (a p v) (b q w) -> c a p v q b w",
                            a=2, p=2, b=48, q=2)[:, :, :, u1, :, :, u2]
                        if r % 2 == 0:
                            nc.scalar.activation(dst, src, AF.Relu,
                                                 bias=bias2[m])
                        else:
                            nc.vector.tensor_scalar(
                                dst, src, bias2[m], 0.0,
                                op0=ALU.add, op1=ALU.max)
                    dsta = bass.AP(d_out[m].tensor,
                                   d_out[m].offset + apair * 1536,
                                   [d_out[m].ap[0], [1, 1536]])
                    nc.sync.dma_start(
                        dsta, stage[m][:].rearrange("c y x -> c (y x)"))


def _build_program():
    import concourse.bacc as bacc
    import concourse.tile as tile
    from concourse import mybir

    dts = {"bf16": mybir.dt.bfloat16, "f32": mybir.dt.float32,
           "i16": mybir.dt.int16}
    nc = bacc.Bacc("TRN2", target_bir_lowering=False, debug=False,
                   num_devices=NCORES)
    ins = {}
    for name, shape, dt in INPUT_SPECS:
        ins[name] = nc.dram_tensor(name, shape, dts[dt],
                                   kind="ExternalInput").ap()
    outs = {"out": nc.dram_tensor("out", [2, 128, 48 * FW],
                                  mybir.dt.bfloat16,
                                  kind="ExternalOutput").ap()}
    with tile.TileContext(nc) as tc:
        device_kernel(tc, outs, ins)
    nc.compile()
    return nc


def _get_program():
    if "nc" not in _CACHE:
        _CACHE["nc"] = _build_program()
    return _CACHE["nc"]


def _core_in_map(hp, core):
    import ml_dtypes
    bf16 = ml_dtypes.bfloat16
    cs = core_slices(hp, core)

    w1t = np.stack([hp["W1"][:, :128].T, hp["W1"][:, 128:].T],
                   axis=1)                            # [128, 2, 64]
    ke1t = np.zeros((Cc, 9, Cc), dtype=np.float32)
    ke2t = np.zeros((Cc, 9, CK), dtype=np.float32)
    for ky in range(3):
        for kx in range(3):
            t = ky * 3 + kx
            ke1t[:, t, :] = hp["W2"][:, :, ky, kx].T
            ke2t[:, t, :] = hp["W3"][:, :, ky, kx].T
    cmats = hp["cmats"].transpose(1, 0, 2)            # [100, 15, 144]
    projt = np.zeros((128, 4, 2, 128), dtype=np.float32)
    for u in range(4):
        for m in range(2):
            blkw = hp["P"][m * 128:(m + 1) * 128, u * 64:(u + 1) * 64].T
            projt[0:64, u, m, :] = blkw
            projt[64:128, u, m, :] = blkw
    gden = np.zeros((CK, 4), dtype=np.float32)
    for u in range(4):
        gden[u * 25:(u + 1) * 25, u] = 1.0
    g2 = np.ascontiguousarray(gden.T)

    packb = np.zeros((128, PB_N), dtype=np.float32)
    packb[:, PB_W1T:PB_W1T + 128] = w1t.reshape(128, 128)
    packb[0:CK, PB_GDEN:PB_GDEN + 4] = gden
    packb[0:4, PB_G2:PB_G2 + CK] = g2
    packb[0:Cc, PB_M1:PB_M1 + R_XIN] = cs["mask_comp"][None, :]
    packb[0:Cc, PB_M2:PB_M2 + 16] = cs["mask_k1"][None, :]

    packk = np.zeros((Cc, PK_N), dtype=np.float32)
    packk[:, PK_KE1:PK_KE1 + 576] = ke1t.reshape(Cc, 576)
    packk[:, PK_KE2:PK_KE2 + 900] = ke2t.reshape(Cc, 900)

    packf = np.zeros((128, 8), dtype=np.float32)
    packf[0:Cc, 0] = hp["bias1"]
    packf[0:Cc, 1] = hp["biask"]
    packf[:, 2] = hp["bias2"][:128]
    packf[:, 3] = hp["bias2"][128:]
    packf[0:CK, 4] = cs["e_top"]
    packf[0:CK, 5] = 1.0 - cs["e_top"]
    packf[0:CK, 6] = cs["e_bot"]
    packf[0:CK, 7] = 1.0 - cs["e_bot"]

    packi = np.full((128, 72), -1, dtype=np.int16)
    packi[:, 0:48] = hp["sidx_a"]
    packi[0:64, 48:72] = hp["sidx_b"]

    return {
        "packf": packf,
        "packi": packi,
        "packb": packb.astype(bf16),
        "packk": packk.astype(bf16),
        "xin": cs["xin"].astype(bf16),
        "cmats": np.ascontiguousarray(cmats).astype(bf16),
        "xt": cs["xt"].astype(bf16),
        "xt2": cs["xt2"].astype(bf16),
        "projt": projt.astype(bf16),
    }


def _gather(results):
    out = np.zeros((N, C, FH, FW), dtype=np.float32)
    for core in range(NCORES):
        n, blk = core // 4, core % 4
        a0 = blk * BLK
        o = np.asarray(results[core]["out"]).astype(np.float32)
        o = o.reshape(2, 128, 48, FW)
        out[n, :128, 4 * a0: 4 * a0 + 48, :] = o[0]
        out[n, 128:, 4 * a0: 4 * a0 + 48, :] = o[1]
    return out


def kernel(**inputs) -> np.ndarray:
    if os.environ.get("CARAFE_EMULATE"):
        return emulate(inputs)
    from concourse.bass_utils import run_bass_kernel_spmd
    hp = host_prep(inputs)
    nc = _get_program()
    in_maps = [_core_in_map(hp, core) for core in range(NCORES)]
    res = run_bass_kernel_spmd(nc, in_maps, list(range(NCORES)),
                               trace=bool(os.environ.get("CARAFE_TRACE")))
    _CACHE["last_results"] = res
    return _gather(res.results)



# revision 34
# speedup vs baseline: 1.0911x; 1.0432x over previous
"""
ContentAwareUpsampling (CARAFE-style) Trainium2 Bass kernel.

x[2,256,48,48] -> out[2,256,192,192]; 8 cores = 2 batches x 4 blocks of 12 rows.

Decomposition (validated bit-close by emulate() against the jax reference):
  branch convs (bf16 matmuls, BN+relu fused into ACT drains)
  softmax via exp (ACT) + group-sum/broadcast matmuls + reciprocal
  T = collapse(bilinear(softmax)) as 9 shifted matmuls vs constant [100,144]
  band matrices B built by GPSIMD local_scatter with constant indices
  einsum  E[c,(u,q,b)] = sum_dy xT[row a+dy].T @ B[dy]     (per row a, parity p)
  proj    F[r] = sum_u P_sub[u].T @ E_sigma[r*64:+64]      (sigma baked in xt)
  pixel-shuffle + BN + relu fused into the ACT drains; DMA out.
"""

import os
import numpy as np

# ---------------------------------------------------------------- constants
N, C, H, W = 2, 256, 48, 48
Cc, CK, UP, K5 = 64, 100, 2, 5
EPS = 1e-5
NCORES = 8
BLK = 12            # lo-res rows per core block
R_XIN = 20          # x rows for branch (a0-4 .. a0+15)
R_XT = 14           # x rows for einsum (a0-1 .. a0+12)
R_KN = 14           # kernel rows (a0-1 .. a0+12)
OH, OW = H * UP, W * UP          # 96
FH, FW = OH * UP, OW * UP        # 192

DYMAP = {0: [-1, -1, 0, 0, 1], 1: [-1, 0, 0, 1, 1]}
BLEND = {0: {-1: 0.25, 0: 0.75}, 1: {0: 0.75, 1: 0.25}}


def _f32(a):
    return np.ascontiguousarray(a, dtype=np.float32)


# ---------------------------------------------------------------- host prep
def _fold_bn(w, g, b, m, v):
    inv = g / np.sqrt(v + EPS)
    return w * inv[:, None], b - m * inv


def build_c2mats():
    """C2[(rho+1)*5 + (s+2)] [100, 144]: D2[b', (p,dy,q,u,dx)] =
    sum_{rho,s} Kn[:, j+rho+1, b'+s+2].T @ C2, with tau = s + dx."""
    cm = np.zeros((15, CK, 144), dtype=np.float32)
    for p in range(2):
        for q in range(2):
            for u in range(4):
                for ky in range(K5):
                    dy = DYMAP[p][ky]
                    for kx in range(K5):
                        dx = DYMAP[q][kx]
                        col = p * 72 + (dy + 1) * 24 + q * 12 + u * 3 + (dx + 1)
                        row = u * 25 + ky * 5 + kx
                        for rho, br in BLEND[p].items():
                            for tau, bc in BLEND[q].items():
                                s = tau - dx
                                w = (rho + 1) * 5 + (s + 2)
                                cm[w, row, col] += br * bc
    return cm


def build_c1():
    """C1 [100, 5 s, 2 p, 3 dy, 24 (q,u,dx)]: fine-band collapse matrices
    with the 0.75 row-blend weight folded in (Kb' = kn[ra]/3 + kn[rb])."""
    c1 = np.zeros((CK, 5, 2, 3, 24), dtype=np.float32)
    for p in range(2):
        for dy_i in range(3):
            dy = dy_i - 1
            for u in range(4):
                for ky in range(K5):
                    if DYMAP[p][ky] != dy:
                        continue
                    for q in range(2):
                        for kx in range(K5):
                            dx = DYMAP[q][kx]
                            for tau, bc in BLEND[q].items():
                                s = tau - dx
                                col = q * 12 + u * 3 + (dx + 1)
                                row = u * 25 + ky * 5 + kx
                                c1[row, s + 2, p, dy_i, col] += 0.75 * bc
    return c1


def build_sidx4():
    """Scatter idx [80, 48]: partition (dy_i*26 + bb), data col
    (p, q, u, dx) -> band position p*192 + u*48 + q*24 + (bb - dx_i)."""
    sidx = np.full((96, 48), -1, dtype=np.int16)
    for dy_i in range(3):
        for bb in range(26):
            for p in range(2):
                for q in range(2):
                    for u in range(4):
                        for dx_i in range(3):
                            bloc = bb - dx_i
                            if 0 <= bloc < 24:
                                t = p * 24 + q * 12 + u * 3 + dx_i
                                sidx[dy_i * 32 + bb, t] = \
                                    p * 192 + u * 48 + q * 24 + bloc
    return sidx


def host_prep(inputs):
    x = _f32(inputs["x"])
    W1, bias1 = _fold_bn(_f32(inputs["compress_w"])[:, :, 0, 0],
                         _f32(inputs["g1"]), _f32(inputs["b1"]),
                         _f32(inputs["m1"]), _f32(inputs["v1"]))
    invk = _f32(inputs["gk"]) / np.sqrt(_f32(inputs["vk"]) + EPS)
    W2 = _f32(inputs["ke1_w"]) * invk[:, None, None, None]
    biask = _f32(inputs["bk"]) - _f32(inputs["mk"]) * invk
    W3 = _f32(inputs["ke2_w"])
    P, bias2 = _fold_bn(_f32(inputs["proj_w"])[:, :, 0, 0],
                        _f32(inputs["g2"]), _f32(inputs["b2"]),
                        _f32(inputs["m2"]), _f32(inputs["v2"]))
    cmats = build_c2mats()
    c1 = build_c1()
    sidx4 = build_sidx4()
    # sigma position p = r*64 + m  ->  channel c = 4m + r
    sigma = np.array([4 * (p % 64) + p // 64 for p in range(256)])
    return dict(x=x, W1=W1, bias1=bias1, W2=W2, biask=biask, W3=W3,
                P=P, bias2=bias2, cmats=cmats, c1=c1, sidx4=sidx4,
                sigma=sigma)


def core_slices(hp, core):
    n, blk = core // 4, core % 4
    a0 = blk * BLK
    x = hp["x"][n]

    xin = np.zeros((2, 128, R_XIN, 52), dtype=np.float32)
    for s in range(R_XIN):
        row = a0 - 4 + s
        if 0 <= row < H:
            xin[0, :, s, 2:50] = x[:128, row, :]
            xin[1, :, s, 2:50] = x[128:, row, :]

    xt = np.zeros((50, R_XT, 2, 128), dtype=np.float32)
    xs = x[hp["sigma"]]
    for s in range(R_XT):
        row = a0 - 1 + s
        if 0 <= row < H:
            xt[1:49, s, 0, :] = xs[:128, row, :].T
            xt[1:49, s, 1, :] = xs[128:, row, :].T

    xt4 = np.zeros((96, 12, 2, 2, 128), dtype=np.float32)
    for dy_i in range(3):
        for half in range(2):
            for bb in range(26):
                xt4[dy_i * 32 + bb, :, half, :, :] = \
                    xt[half * 24 + bb, dy_i:dy_i + 12, :, :]

    e_top = 1.0 if a0 == 0 else 0.0
    e_bot = 1.0 if a0 + BLK == H else 0.0
    mask_comp = np.array([1.0 if 0 <= a0 - 4 + s < H else 0.0
                          for s in range(R_XIN)], dtype=np.float32)
    mask_k1 = np.array([1.0 if 0 <= a0 - 2 + s < H else 0.0
                        for s in range(16)], dtype=np.float32)
    return dict(a0=a0, n=n, xin=xin, xt=xt, xt4=xt4, e_top=e_top,
                e_bot=e_bot, mask_comp=mask_comp, mask_k1=mask_k1)


# ---------------------------------------------------------------- emulation
def emulate_core(hp, cs):
    xin, xt = cs["xin"], cs["xt"]
    W1, W2, W3 = hp["W1"], hp["W2"], hp["W3"]

    xin_flat = np.concatenate([xin[0], xin[1]], axis=0)
    comp = np.zeros((Cc, R_XIN, 52), dtype=np.float32)
    comp[:, :, 2:50] = np.maximum(
        np.einsum("oc,csw->osw", W1, xin_flat[:, :, 2:50])
        + hp["bias1"][:, None, None], 0.0)
    comp *= cs["mask_comp"][None, :, None]

    k1 = np.zeros((Cc, 16, 50), dtype=np.float32)
    acc = np.zeros((Cc, 16, 48), dtype=np.float32)
    for ky in range(3):
        for kx in range(3):
            sh = comp[:, ky * 2: ky * 2 + 16, kx * 2: kx * 2 + 48]
            acc += np.einsum("oi,isw->osw", W2[:, :, ky, kx], sh)
    k1[:, :, 1:49] = np.maximum(acc + hp["biask"][:, None, None], 0.0)
    k1 *= cs["mask_k1"][None, :, None]

    kc = np.zeros((CK, R_KN, 48), dtype=np.float32)
    for ky in range(3):
        for kx in range(3):
            kc += np.einsum("oi,isw->osw", W3[:, :, ky, kx],
                            k1[:, ky: ky + 14, kx: kx + 48])

    e = np.exp(kc.reshape(4, 25, R_KN, 48))
    kn_int = (e / e.sum(axis=1, keepdims=True)).reshape(CK, R_KN, 48)

    # Kn [100, 14, 54]: col i <-> kernel pixel b = i - 3; interior i 3..50
    Kn = np.zeros((CK, R_KN, 54), dtype=np.float32)
    Kn[:, :, 3:51] = kn_int
    Kn[:, :, 2] = Kn[:, :, 3]
    Kn[:, :, 51] = Kn[:, :, 50]
    if cs["e_top"] == 1.0:
        Kn[:, 0, :] = Kn[:, 1, :]
    if cs["e_bot"] == 1.0:
        Kn[:, 13, :] = Kn[:, 12, :]

    cmats, P = hp["cmats"], hp["P"]
    scat_idx = np.full((64, 72), -1, dtype=np.int16)
    for bp in range(50):
        for dy_i in range(3):
            for q in range(2):
                for u in range(4):
                    for dx_i in range(3):
                        t = dy_i * 24 + q * 12 + u * 3 + dx_i
                        b = bp - 1 - (dx_i - 1)
                        if 0 <= b < 48:
                            scat_idx[bp, t] = dy_i * 384 + u * 96 + q * 48 + b
    out = np.zeros((C, 4 * BLK, FW), dtype=np.float32)
    for j in range(BLK):
        # D2 [50, (p, dy, q, u, dx)]
        D2 = np.zeros((50, 144), dtype=np.float32)
        for rho in (-1, 0, 1):
            for s in (-2, -1, 0, 1, 2):
                w = (rho + 1) * 5 + (s + 2)
                D2 += Kn[:, j + rho + 1, s + 2: s + 52].T @ cmats[w]
        for p in range(2):
            D = np.zeros((64, 72), dtype=np.float32)
            D[0:50] = D2[:, p * 72:(p + 1) * 72]
            B = np.zeros((64, 1152), dtype=np.float32)
            for bp in range(64):
                for t in range(72):
                    ix = scat_idx[bp, t]
                    if ix >= 0:
                        B[bp, ix] = D[bp, t]
            E = np.zeros((256, 384), dtype=np.float32)
            for ch in range(2):
                acc = np.zeros((128, 384), dtype=np.float32)
                for dy_i in range(3):
                    xsl = xt[0:50, j + dy_i, ch, :]
                    acc += xsl.T @ B[0:50, dy_i * 384:(dy_i + 1) * 384]
                E[ch * 128:(ch + 1) * 128] = acc
            for r in range(4):
                u1, u2 = r // 2, r % 2
                F = np.zeros((C, 96), dtype=np.float32)
                for u in range(4):
                    F += P[:, u * 64:(u + 1) * 64] @ \
                        E[r * 64:(r + 1) * 64, u * 96:(u + 1) * 96]
                F = np.maximum(F + hp["bias2"][:, None], 0.0)
                Y = 4 * j + 2 * p + u1
                Fq = F.reshape(C, 2, 48)
                for q in range(2):
                    out[:, Y, u2 + 2 * q::4] = Fq[:, q, :]
    return out


def emulate(inputs):
    hp = host_prep(inputs)
    out = np.zeros((N, C, FH, FW), dtype=np.float32)
    for core in range(NCORES):
        cs = core_slices(hp, core)
        n, a0 = cs["n"], cs["a0"]
        out[n, :, 4 * a0: 4 * a0 + 48, :] = emulate_core(hp, cs)
    return out


# ---------------------------------------------------------------- device
_CACHE = {}

# packb column layout (tiny branch-critical bf16 pack, first DMA)
PB_W1T = 0          # [128, 2*64]   -> 0:128
PB_GDEN = 128       # [100, 4]      -> 128:132
PB_G2 = 132         # [4, 100]      -> 132:232
PB_M1 = 232         # [64, 20]      -> 232:252
PB_M2 = 252         # [64, 16]      -> 252:268
PB_N = 268
# packk column layout (conv weights, loads behind xin)
PK_KE1 = 0          # [64, 9*64]    -> 0:576
PK_KE2 = 576        # [64, 9*100]   -> 576:1476
PK_N = 1476

INPUT_SPECS = [
    # branch-critical first (DMA order follows list order)
    ("packb", [128, PB_N], "bf16"),
    ("xin", [2, 128, R_XIN, 52], "bf16"),
    ("packf", [128, 8], "f32"),
    ("packk", [Cc, PK_N], "bf16"),
    ("sidx4", [96, 48], "i16"),
    ("c1", [CK, 5, 2, 3, 24], "bf16"),
    ("xt4", [96, 12, 2, 2, 128], "bf16"),
    ("projt", [128, 4, 2, 128], "bf16"),
]


def device_kernel(tc, outs, ins):
    """Emit the kernel. outs/ins: dicts name -> bass.AP (DRAM)."""
    from contextlib import ExitStack
    import concourse.bass as bass
    from concourse import mybir

    f32 = mybir.dt.float32
    bf16 = mybir.dt.bfloat16
    i16 = mybir.dt.int16
    AF = mybir.ActivationFunctionType
    ALU = mybir.AluOpType
    nc = tc.nc
    d_out = outs["out"]

    with ExitStack() as ctx:
        sing = ctx.enter_context(tc.tile_pool(name="sing", bufs=1))

        # ---- load inputs/constants (critical-path first, bulky ones later)
        sb = {}
        dts = {"bf16": bf16, "f32": f32, "i16": i16}
        deferred = ("c1", "xt4", "projt")
        for name, shape, dt in INPUT_SPECS:
            t = sing.tile(shape[:] if name != "xin" else [128, R_XIN, 52],
                          dts[dt], tag=name)
            if name == "xin":
                sb["xin0"] = t
                t2 = sing.tile([128, R_XIN, 52], bf16, tag="xin1")
                sb["xin1"] = t2
                nc.sync.dma_start(t[:], ins["xin"][0])
                nc.sync.dma_start(t2[:], ins["xin"][1])
            else:
                sb[name] = t
                if name not in deferred:
                    nc.sync.dma_start(t[:], ins[name])

        # typed views into the packs
        pb, pf = sb["packb"], sb["packf"]
        pk = sb["packk"]
        sb["w1t"] = pb[:, PB_W1T:PB_W1T + 128].rearrange(
            "p (c k) -> p c k", c=2)
        sb["ke1t"] = pk[0:Cc, PK_KE1:PK_KE1 + 576].rearrange(
            "p (t k) -> p t k", t=9)
        sb["ke2t"] = pk[0:Cc, PK_KE2:PK_KE2 + 900].rearrange(
            "p (t k) -> p t k", t=9)
        sb["gden"] = pb[0:CK, PB_GDEN:PB_GDEN + 4]
        sb["g2"] = pb[0:4, PB_G2:PB_G2 + CK]
        # only the halo rows can fall outside the image: comp rows 0:4 /
        # 16:20, k1 rows 0:2 / 14:16.  Interior rows always have mask 1.
        m1a = pb[0:Cc, PB_M1:PB_M1 + 4].unsqueeze(2).broadcast_to(
            (Cc, 4, 52))
        m1b = pb[0:Cc, PB_M1 + 16:PB_M1 + 20].unsqueeze(2).broadcast_to(
            (Cc, 4, 52))
        m2a = pb[0:Cc, PB_M2:PB_M2 + 2].unsqueeze(2).broadcast_to(
            (Cc, 2, 50))
        m2b = pb[0:Cc, PB_M2 + 14:PB_M2 + 16].unsqueeze(2).broadcast_to(
            (Cc, 2, 50))
        sb["bias1"] = pf[0:Cc, 0:1]
        sb["biask"] = pf[0:Cc, 1:2]
        bias2 = [pf[:, 2:3], pf[:, 3:4]]
        sb["etop"] = pf[0:CK, 4:6]
        sb["ebot"] = pf[0:CK, 6:8]

        # persistent activations
        comp = sing.tile([Cc, R_XIN, 52], bf16)
        k1 = sing.tile([Cc, 16, 50], bf16)
        exps = sing.tile([CK, 672], bf16)
        recs = sing.tile([4, 672], bf16)
        kn = sing.tile([CK, R_KN, 54], bf16)
        nc.vector.memset(comp[:], 0.0)
        nc.vector.memset(k1[:], 0.0)
        nc.vector.memset(kn[:], 0.0)

        # ---- branch phase
        with tc.tile_pool(name="brp", bufs=2, space="PSUM") as brp:
            for h in range(2):
                ps = brp.tile([Cc, 10 * 48], f32, tag="psc")
                for c in range(2):
                    nc.tensor.matmul(
                        ps[:], sb["w1t"][:, c, :],
                        sb[f"xin{c}"][:, h * 10:(h + 1) * 10, 2:50],
                        start=(c == 0), stop=(c == 1))
                nc.scalar.activation(
                    comp[:, h * 10:(h + 1) * 10, 2:50],
                    ps[:].rearrange("c (r w) -> c r w", w=48),
                    AF.Relu, bias=sb["bias1"][:])
                if h == 0:
                    nc.vector.tensor_mul(comp[:, 0:4, :], comp[:, 0:4, :],
                                         m1a)
                else:
                    nc.vector.tensor_mul(comp[:, 16:20, :],
                                         comp[:, 16:20, :], m1b)
            for name in deferred:
                nc.sync.dma_start(sb[name][:], ins[name])
            for h in range(2):
                ps = brp.tile([Cc, 8 * 48], f32, tag="psk1")
                for ky in range(3):
                    for kx in range(3):
                        t = ky * 3 + kx
                        nc.tensor.matmul(
                            ps[:], sb["ke1t"][:, t, :],
                            comp[:, h * 8 + ky * 2: h * 8 + ky * 2 + 8,
                                 kx * 2: kx * 2 + 48],
                            start=(t == 0), stop=(t == 8))
                nc.scalar.activation(
                    k1[:, h * 8:(h + 1) * 8, 1:49],
                    ps[:].rearrange("c (r w) -> c r w", w=48),
                    AF.Relu, bias=sb["biask"][:])
                if h == 0:
                    nc.vector.tensor_mul(k1[:, 0:2, :], k1[:, 0:2, :], m2a)
                else:
                    nc.vector.tensor_mul(k1[:, 14:16, :], k1[:, 14:16, :],
                                         m2b)
            for h in range(2):
                ps = brp.tile([CK, 7 * 48], f32, tag="psk2", bufs=2)
                for ky in range(3):
                    for kx in range(3):
                        t = ky * 3 + kx
                        nc.tensor.matmul(
                            ps[:], sb["ke2t"][:, t, :],
                            k1[:, h * 7 + ky: h * 7 + ky + 7, kx: kx + 48],
                            start=(t == 0), stop=(t == 8))
                nc.scalar.activation(exps[:, h * 336:(h + 1) * 336], ps[:],
                                     AF.Exp)
            for h in range(2):
                psd = brp.tile([4, 336], f32, tag="psc")
                nc.tensor.matmul(psd[:], sb["gden"][:],
                                 exps[:, h * 336:(h + 1) * 336],
                                 start=True, stop=True)
                with nc.allow_low_precision(reason="bf16 softmax pipeline"):
                    nc.vector.reciprocal(recs[:, h * 336:(h + 1) * 336],
                                         psd[:])
            for h in range(2):
                psb = brp.tile([CK, 336], f32, tag="psc")
                nc.tensor.matmul(psb[:], sb["g2"][:],
                                 recs[:, h * 336:(h + 1) * 336],
                                 start=True, stop=True)
                nc.vector.tensor_mul(
                    kn[:, h * 7:(h + 1) * 7, 3:51],
                    exps[:, h * 336:(h + 1) * 336].rearrange(
                        "c (r w) -> c r w", w=48),
                    psb[:].rearrange("c (r w) -> c r w", w=48))
                rs = slice(h * 7, (h + 1) * 7)
                nc.vector.tensor_copy(kn[:, rs, 2:3], kn[:, rs, 3:4])
                nc.vector.tensor_copy(kn[:, rs, 51:52], kn[:, rs, 50:51])
                if h == 0:
                    nc.vector.tensor_scalar_mul(kn[:, 0, :], kn[:, 0, :],
                                                sb["etop"][:, 1:2])
                    nc.vector.scalar_tensor_tensor(
                        kn[:, 0, :], kn[:, 1, :], sb["etop"][:, 0:1],
                        kn[:, 0, :], op0=ALU.mult, op1=ALU.add)
                else:
                    nc.vector.tensor_scalar_mul(kn[:, 13, :], kn[:, 13, :],
                                                sb["ebot"][:, 1:2])
                    nc.vector.scalar_tensor_tensor(
                        kn[:, 13, :], kn[:, 12, :], sb["ebot"][:, 0:1],
                        kn[:, 13, :], op0=ALU.mult, op1=ALU.add)

        # ---- main loop
        kbpool = ctx.enter_context(tc.tile_pool(name="kb", bufs=3))
        dpool = ctx.enter_context(tc.tile_pool(name="dsb", bufs=4))
        bpool = ctx.enter_context(tc.tile_pool(name="bb", bufs=6))
        epool = ctx.enter_context(tc.tile_pool(name="esb", bufs=3))
        spool = ctx.enter_context(tc.tile_pool(name="stage", bufs=4))
        pp_t = ctx.enter_context(tc.tile_pool(name="pp_t", bufs=2, space="PSUM"))
        pp_e = ctx.enter_context(tc.tile_pool(name="pp_e", bufs=3, space="PSUM"))
        pp_f = ctx.enter_context(tc.tile_pool(name="pp_f", bufs=3, space="PSUM"))

        esb = {}
        stage = {}
        bts = {}

        def emit_tdb(j):
            # row-blend + fine-band T collapse + band scatter for row j
            kb = kbpool.tile([CK, 2, 54], bf16, tag="kb", name=f"kb{j}")
            nc.vector.scalar_tensor_tensor(
                kb[:], kn[:, j:j + 3:2, :], 1.0 / 3.0,
                kn[:, j + 1:j + 2, :].broadcast_to((CK, 2, 54)),
                op0=ALU.mult, op1=ALU.add)
            for half in range(2):
                pst = pp_t.tile([96, 48], f32, tag="t",
                                name=f"pst{j}_{half}")
                for dy_i in range(3):
                    for p in range(2):
                        for si in range(5):
                            w0 = half * 24 + si
                            nc.tensor.matmul(
                                pst[dy_i * 32: dy_i * 32 + 26,
                                    p * 24:(p + 1) * 24],
                                kb[:, p, w0:w0 + 26],
                                sb["c1"][:, si, p, dy_i, :],
                                start=(si == 0), stop=(si == 4))
                dsb = dpool.tile([96, 48], bf16, tag="d",
                                 name=f"dsb{j}_{half}")
                nc.scalar.copy(dsb[:], pst[:])
                b4 = bpool.tile([96, 384], bf16, tag="b",
                                name=f"b4_{j}_{half}")
                nc.gpsimd.local_scatter(
                    b4[:], dsb[:], sb["sidx4"][:], channels=96,
                    num_elems=384, num_idxs=48)
                bts[(j, half)] = b4

        emit_tdb(0)
        emit_tdb(1)
        for j in range(BLK):
            if j + 2 < BLK:
                emit_tdb(j + 2)
            if j % 2 == 0:
                esb = {c: epool.tile([128, 2, 4, 2, 2, 48], bf16,
                                     tag=f"e{c}", name=f"esb{c}")
                       for c in range(2)}
                stage = {m: spool.tile([128, 8, 192], bf16,
                                       tag=f"s{m}", name=f"stage{m}")
                         for m in range(2)}
            b4h = [bts.pop((j, 0)), bts.pop((j, 1))]
            for p in range(2):
                for c in range(2):
                    pse = pp_e.tile([128, 384], f32, tag="e")
                    for half in range(2):
                        nc.tensor.matmul(
                            pse[:, half * 192:(half + 1) * 192],
                            sb["xt4"][0:90, j, half, c, :],
                            b4h[half][0:90, p * 192:(p + 1) * 192],
                            start=True, stop=True)
                    for half in range(2):
                        src = pse[:, half * 192:(half + 1) * 192].rearrange(
                            "c (u q b) -> c u q b", u=4, q=2)
                        dst = esb[c][:, j % 2, :, p, :,
                                     half * 24:half * 24 + 24]
                        if (p + c + half) % 2 == 0:
                            nc.vector.tensor_copy(dst, src)
                        else:
                            nc.scalar.copy(dst, src)

            if j % 2 == 1:
                apair = j // 2
                for m in range(2):
                    for r in range(4):
                        u1, u2 = r // 2, r % 2
                        base = (r % 2) * 64
                        psf = pp_f.tile([128, 2 * 192], f32, tag="f")
                        for u in range(4):
                            nc.tensor.matmul(
                                psf[:], sb["projt"][base:base + 64, u, m, :],
                                esb[r // 2][base:base + 64, :, u, :, :, :],
                                start=(u == 0), stop=(u == 3))
                        # psf cols = (a2, p, q, b); out row Y = 4*a2+2p+u1,
                        # col X = u2 + 2q + 4b
                        src = psf[:].rearrange(
                            "c (a p q b) -> c a p q b", a=2, p=2, q=2)
                        dst = stage[m][:].rearrange(
                            "c (a p v) (b q w) -> c a p v q b w",
                            a=2, p=2, b=48, q=2)[:, :, :, u1, :, :, u2]
                        if r % 2 == 0:
                            nc.scalar.activation(dst, src, AF.Relu,
                                                 bias=bias2[m])
                        else:
                            nc.vector.tensor_scalar(
                                dst, src, bias2[m], 0.0,
                                op0=ALU.add, op1=ALU.max)
                    dsta = bass.AP(d_out[m].tensor,
                                   d_out[m].offset + apair * 1536,
                                   [d_out[m].ap[0], [1, 1536]])
                    nc.sync.dma_start(
                        dsta, stage[m][:].rearrange("c y x -> c (y x)"))


def _build_program():
    import concourse.bacc as bacc
    import concourse.tile as tile
    from concourse import mybir

    dts = {"bf16": mybir.dt.bfloat16, "f32": mybir.dt.float32,
           "i16": mybir.dt.int16}
    nc = bacc.Bacc("TRN2", target_bir_lowering=False, debug=False,
                   num_devices=NCORES)
    ins = {}
    for name, shape, dt in INPUT_SPECS:
        ins[name] = nc.dram_tensor(name, shape, dts[dt],
                                   kind="ExternalInput").ap()
    outs = {"out": nc.dram_tensor("out", [2, 128, 48 * FW],
                                  mybir.dt.bfloat16,
                                  kind="ExternalOutput").ap()}
    with tile.TileContext(nc) as tc:
        device_kernel(tc, outs, ins)
    nc.compile()
    return nc


def _get_program():
    if "nc" not in _CACHE:
        _CACHE["nc"] = _build_program()
    return _CACHE["nc"]


def _core_in_map(hp, core):
    import ml_dtypes
    bf16 = ml_dtypes.bfloat16
    cs = core_slices(hp, core)

    w1t = np.stack([hp["W1"][:, :128].T, hp["W1"][:, 128:].T],
                   axis=1)                            # [128, 2, 64]
    ke1t = np.zeros((Cc, 9, Cc), dtype=np.float32)
    ke2t = np.zeros((Cc, 9, CK), dtype=np.float32)
    for ky in range(3):
        for kx in range(3):
            t = ky * 3 + kx
            ke1t[:, t, :] = hp["W2"][:, :, ky, kx].T
            ke2t[:, t, :] = hp["W3"][:, :, ky, kx].T
    projt = np.zeros((128, 4, 2, 128), dtype=np.float32)
    for u in range(4):
        for m in range(2):
            blkw = hp["P"][m * 128:(m + 1) * 128, u * 64:(u + 1) * 64].T
            projt[0:64, u, m, :] = blkw
            projt[64:128, u, m, :] = blkw
    gden = np.zeros((CK, 4), dtype=np.float32)
    for u in range(4):
        gden[u * 25:(u + 1) * 25, u] = 1.0
    g2 = np.ascontiguousarray(gden.T)

    packb = np.zeros((128, PB_N), dtype=np.float32)
    packb[:, PB_W1T:PB_W1T + 128] = w1t.reshape(128, 128)
    packb[0:CK, PB_GDEN:PB_GDEN + 4] = gden
    packb[0:4, PB_G2:PB_G2 + CK] = g2
    packb[0:Cc, PB_M1:PB_M1 + R_XIN] = cs["mask_comp"][None, :]
    packb[0:Cc, PB_M2:PB_M2 + 16] = cs["mask_k1"][None, :]

    packk = np.zeros((Cc, PK_N), dtype=np.float32)
    packk[:, PK_KE1:PK_KE1 + 576] = ke1t.reshape(Cc, 576)
    packk[:, PK_KE2:PK_KE2 + 900] = ke2t.reshape(Cc, 900)

    packf = np.zeros((128, 8), dtype=np.float32)
    packf[0:Cc, 0] = hp["bias1"]
    packf[0:Cc, 1] = hp["biask"]
    packf[:, 2] = hp["bias2"][:128]
    packf[:, 3] = hp["bias2"][128:]
    packf[0:CK, 4] = cs["e_top"]
    packf[0:CK, 5] = 1.0 - cs["e_top"]
    packf[0:CK, 6] = cs["e_bot"]
    packf[0:CK, 7] = 1.0 - cs["e_bot"]

    return {
        "packf": packf,
        "packb": packb.astype(bf16),
        "packk": packk.astype(bf16),
        "xin": cs["xin"].astype(bf16),
        "sidx4": hp["sidx4"],
        "c1": hp["c1"].astype(bf16),
        "xt4": cs["xt4"].astype(bf16),
        "projt": projt.astype(bf16),
    }


def _gather(results):
    out = np.zeros((N, C, FH, FW), dtype=np.float32)
    for core in range(NCORES):
        n, blk = core // 4, core % 4
        a0 = blk * BLK
        o = np.asarray(results[core]["out"]).astype(np.float32)
        o = o.reshape(2, 128, 48, FW)
        out[n, :128, 4 * a0: 4 * a0 + 48, :] = o[0]
        out[n, 128:, 4 * a0: 4 * a0 + 48, :] = o[1]
    return out


def kernel(**inputs) -> np.ndarray:
    if os.environ.get("CARAFE_EMULATE"):
        return emulate(inputs)
    from concourse.bass_utils import run_bass_kernel_spmd
    hp = host_prep(inputs)
    nc = _get_program()
    in_maps = [_core_in_map(hp, core) for core in range(NCORES)]
    res = run_bass_kernel_spmd(nc, in_maps, list(range(NCORES)),
                               trace=bool(os.environ.get("CARAFE_TRACE")))
    _CACHE["last_results"] = res
    return _gather(res.results)



# revision 40
# speedup vs baseline: 1.2735x; 1.1671x over previous
"""
ContentAwareUpsampling (CARAFE-style) Trainium2 Bass kernel.

x[2,256,48,48] -> out[2,256,192,192]; 8 cores = 2 batches x 4 blocks of 12 rows.

Decomposition (validated bit-close by emulate() against the jax reference):
  branch convs (bf16 matmuls, BN+relu fused into ACT drains)
  softmax via exp (ACT) + group-sum/broadcast matmuls + reciprocal
  T = collapse(bilinear(softmax)) as 9 shifted matmuls vs constant [100,144]
  band matrices B built by GPSIMD local_scatter with constant indices
  einsum  E[c,(u,q,b)] = sum_dy xT[row a+dy].T @ B[dy]     (per row a, parity p)
  proj    F[r] = sum_u P_sub[u].T @ E_sigma[r*64:+64]      (sigma baked in xt)
  pixel-shuffle + BN + relu fused into the ACT drains; DMA out.
"""

import os
import numpy as np

# ---------------------------------------------------------------- constants
N, C, H, W = 2, 256, 48, 48
Cc, CK, UP, K5 = 64, 100, 2, 5
EPS = 1e-5
NCORES = 8
BLK = 12            # lo-res rows per core block
R_XIN = 20          # x rows for branch (a0-4 .. a0+15)
R_XT = 14           # x rows for einsum (a0-1 .. a0+12)
R_KN = 14           # kernel rows (a0-1 .. a0+12)
OH, OW = H * UP, W * UP          # 96
FH, FW = OH * UP, OW * UP        # 192

DYMAP = {0: [-1, -1, 0, 0, 1], 1: [-1, 0, 0, 1, 1]}
BLEND = {0: {-1: 0.25, 0: 0.75}, 1: {0: 0.75, 1: 0.25}}


def _f32(a):
    return np.ascontiguousarray(a, dtype=np.float32)


# ---------------------------------------------------------------- host prep
def _fold_bn(w, g, b, m, v):
    inv = g / np.sqrt(v + EPS)
    return w * inv[:, None], b - m * inv


def build_c2mats():
    """C2[(rho+1)*5 + (s+2)] [100, 144]: D2[b', (p,dy,q,u,dx)] =
    sum_{rho,s} Kn[:, j+rho+1, b'+s+2].T @ C2, with tau = s + dx."""
    cm = np.zeros((15, CK, 144), dtype=np.float32)
    for p in range(2):
        for q in range(2):
            for u in range(4):
                for ky in range(K5):
                    dy = DYMAP[p][ky]
                    for kx in range(K5):
                        dx = DYMAP[q][kx]
                        col = p * 72 + (dy + 1) * 24 + q * 12 + u * 3 + (dx + 1)
                        row = u * 25 + ky * 5 + kx
                        for rho, br in BLEND[p].items():
                            for tau, bc in BLEND[q].items():
                                s = tau - dx
                                w = (rho + 1) * 5 + (s + 2)
                                cm[w, row, col] += br * bc
    return cm


def build_c1():
    """C1 [100, 5 s, 2 p, 3 dy, 24 (q,u,dx)]: fine-band collapse matrices
    with the 0.75 row-blend weight folded in (Kb' = kn[ra]/3 + kn[rb])."""
    c1 = np.zeros((CK, 5, 2, 3, 24), dtype=np.float32)
    for p in range(2):
        for dy_i in range(3):
            dy = dy_i - 1
            for u in range(4):
                for ky in range(K5):
                    if DYMAP[p][ky] != dy:
                        continue
                    for q in range(2):
                        for kx in range(K5):
                            dx = DYMAP[q][kx]
                            for tau, bc in BLEND[q].items():
                                s = tau - dx
                                col = q * 12 + u * 3 + (dx + 1)
                                row = u * 25 + ky * 5 + kx
                                c1[row, s + 2, p, dy_i, col] += 0.75 * bc
    return c1


def build_sidx4():
    """Scatter idx [80, 48]: partition (dy_i*26 + bb), data col
    (p, q, u, dx) -> band position p*192 + u*48 + q*24 + (bb - dx_i)."""
    sidx = np.full((96, 48), -1, dtype=np.int16)
    for dy_i in range(3):
        for bb in range(26):
            for p in range(2):
                for q in range(2):
                    for u in range(4):
                        for dx_i in range(3):
                            bloc = bb - dx_i
                            if 0 <= bloc < 24:
                                t = p * 24 + q * 12 + u * 3 + dx_i
                                sidx[dy_i * 32 + bb, t] = \
                                    p * 192 + u * 48 + q * 24 + bloc
    return sidx


def host_prep(inputs):
    x = _f32(inputs["x"])
    W1, bias1 = _fold_bn(_f32(inputs["compress_w"])[:, :, 0, 0],
                         _f32(inputs["g1"]), _f32(inputs["b1"]),
                         _f32(inputs["m1"]), _f32(inputs["v1"]))
    invk = _f32(inputs["gk"]) / np.sqrt(_f32(inputs["vk"]) + EPS)
    W2 = _f32(inputs["ke1_w"]) * invk[:, None, None, None]
    biask = _f32(inputs["bk"]) - _f32(inputs["mk"]) * invk
    W3 = _f32(inputs["ke2_w"])
    P, bias2 = _fold_bn(_f32(inputs["proj_w"])[:, :, 0, 0],
                        _f32(inputs["g2"]), _f32(inputs["b2"]),
                        _f32(inputs["m2"]), _f32(inputs["v2"]))
    cmats = build_c2mats()
    c1 = build_c1()
    sidx4 = build_sidx4()
    # sigma position p = r*64 + m  ->  channel c = 4m + r
    sigma = np.array([4 * (p % 64) + p // 64 for p in range(256)])
    return dict(x=x, W1=W1, bias1=bias1, W2=W2, biask=biask, W3=W3,
                P=P, bias2=bias2, cmats=cmats, c1=c1, sidx4=sidx4,
                sigma=sigma)


def core_slices(hp, core):
    n, blk = core // 4, core % 4
    a0 = blk * BLK
    x = hp["x"][n]

    xin = np.zeros((2, 128, R_XIN, 52), dtype=np.float32)
    for s in range(R_XIN):
        row = a0 - 4 + s
        if 0 <= row < H:
            xin[0, :, s, 2:50] = x[:128, row, :]
            xin[1, :, s, 2:50] = x[128:, row, :]

    xt = np.zeros((50, R_XT, 2, 128), dtype=np.float32)
    xs = x[hp["sigma"]]
    for s in range(R_XT):
        row = a0 - 1 + s
        if 0 <= row < H:
            xt[1:49, s, 0, :] = xs[:128, row, :].T
            xt[1:49, s, 1, :] = xs[128:, row, :].T

    xt4 = np.zeros((96, 12, 2, 2, 128), dtype=np.float32)
    for dy_i in range(3):
        for half in range(2):
            for bb in range(26):
                xt4[dy_i * 32 + bb, :, half, :, :] = \
                    xt[half * 24 + bb, dy_i:dy_i + 12, :, :]

    e_top = 1.0 if a0 == 0 else 0.0
    e_bot = 1.0 if a0 + BLK == H else 0.0
    mask_comp = np.array([1.0 if 0 <= a0 - 4 + s < H else 0.0
                          for s in range(R_XIN)], dtype=np.float32)
    mask_k1 = np.array([1.0 if 0 <= a0 - 2 + s < H else 0.0
                        for s in range(16)], dtype=np.float32)
    return dict(a0=a0, n=n, xin=xin, xt=xt, xt4=xt4, e_top=e_top,
                e_bot=e_bot, mask_comp=mask_comp, mask_k1=mask_k1)


# ---------------------------------------------------------------- emulation
def emulate_core(hp, cs):
    xin, xt = cs["xin"], cs["xt"]
    W1, W2, W3 = hp["W1"], hp["W2"], hp["W3"]

    xin_flat = np.concatenate([xin[0], xin[1]], axis=0)
    comp = np.zeros((Cc, R_XIN, 52), dtype=np.float32)
    comp[:, :, 2:50] = np.maximum(
        np.einsum("oc,csw->osw", W1, xin_flat[:, :, 2:50])
        + hp["bias1"][:, None, None], 0.0)
    comp *= cs["mask_comp"][None, :, None]

    k1 = np.zeros((Cc, 16, 50), dtype=np.float32)
    acc = np.zeros((Cc, 16, 48), dtype=np.float32)
    for ky in range(3):
        for kx in range(3):
            sh = comp[:, ky * 2: ky * 2 + 16, kx * 2: kx * 2 + 48]
            acc += np.einsum("oi,isw->osw", W2[:, :, ky, kx], sh)
    k1[:, :, 1:49] = np.maximum(acc + hp["biask"][:, None, None], 0.0)
    k1 *= cs["mask_k1"][None, :, None]

    kc = np.zeros((CK, R_KN, 48), dtype=np.float32)
    for ky in range(3):
        for kx in range(3):
            kc += np.einsum("oi,isw->osw", W3[:, :, ky, kx],
                            k1[:, ky: ky + 14, kx: kx + 48])

    e = np.exp(kc.reshape(4, 25, R_KN, 48))
    kn_int = (e / e.sum(axis=1, keepdims=True)).reshape(CK, R_KN, 48)

    # Kn [100, 14, 54]: col i <-> kernel pixel b = i - 3; interior i 3..50
    Kn = np.zeros((CK, R_KN, 54), dtype=np.float32)
    Kn[:, :, 3:51] = kn_int
    Kn[:, :, 2] = Kn[:, :, 3]
    Kn[:, :, 51] = Kn[:, :, 50]
    if cs["e_top"] == 1.0:
        Kn[:, 0, :] = Kn[:, 1, :]
    if cs["e_bot"] == 1.0:
        Kn[:, 13, :] = Kn[:, 12, :]

    cmats, P = hp["cmats"], hp["P"]
    scat_idx = np.full((64, 72), -1, dtype=np.int16)
    for bp in range(50):
        for dy_i in range(3):
            for q in range(2):
                for u in range(4):
                    for dx_i in range(3):
                        t = dy_i * 24 + q * 12 + u * 3 + dx_i
                        b = bp - 1 - (dx_i - 1)
                        if 0 <= b < 48:
                            scat_idx[bp, t] = dy_i * 384 + u * 96 + q * 48 + b
    out = np.zeros((C, 4 * BLK, FW), dtype=np.float32)
    for j in range(BLK):
        # D2 [50, (p, dy, q, u, dx)]
        D2 = np.zeros((50, 144), dtype=np.float32)
        for rho in (-1, 0, 1):
            for s in (-2, -1, 0, 1, 2):
                w = (rho + 1) * 5 + (s + 2)
                D2 += Kn[:, j + rho + 1, s + 2: s + 52].T @ cmats[w]
        for p in range(2):
            D = np.zeros((64, 72), dtype=np.float32)
            D[0:50] = D2[:, p * 72:(p + 1) * 72]
            B = np.zeros((64, 1152), dtype=np.float32)
            for bp in range(64):
                for t in range(72):
                    ix = scat_idx[bp, t]
                    if ix >= 0:
                        B[bp, ix] = D[bp, t]
            E = np.zeros((256, 384), dtype=np.float32)
            for ch in range(2):
                acc = np.zeros((128, 384), dtype=np.float32)
                for dy_i in range(3):
                    xsl = xt[0:50, j + dy_i, ch, :]
                    acc += xsl.T @ B[0:50, dy_i * 384:(dy_i + 1) * 384]
                E[ch * 128:(ch + 1) * 128] = acc
            for r in range(4):
                u1, u2 = r // 2, r % 2
                F = np.zeros((C, 96), dtype=np.float32)
                for u in range(4):
                    F += P[:, u * 64:(u + 1) * 64] @ \
                        E[r * 64:(r + 1) * 64, u * 96:(u + 1) * 96]
                F = np.maximum(F + hp["bias2"][:, None], 0.0)
                Y = 4 * j + 2 * p + u1
                Fq = F.reshape(C, 2, 48)
                for q in range(2):
                    out[:, Y, u2 + 2 * q::4] = Fq[:, q, :]
    return out


def emulate(inputs):
    hp = host_prep(inputs)
    out = np.zeros((N, C, FH, FW), dtype=np.float32)
    for core in range(NCORES):
        cs = core_slices(hp, core)
        n, a0 = cs["n"], cs["a0"]
        out[n, :, 4 * a0: 4 * a0 + 48, :] = emulate_core(hp, cs)
    return out


# ---------------------------------------------------------------- device
_CACHE = {}

# packb column layout (tiny branch-critical bf16 pack, first DMA)
PB_W1T = 0          # [128, 2*64]   -> 0:128
PB_GDEN = 128       # [100, 4]      -> 128:132
PB_G2 = 132         # [4, 100]      -> 132:232
PB_M1 = 232         # [64, 20]      -> 232:252
PB_M2 = 252         # [64, 16]      -> 252:268
PB_N = 268
# packk column layout (conv weights, loads behind xin)
PK_KE1 = 0          # [64, 9*64]    -> 0:576
PK_KE2 = 576        # [64, 9*100]   -> 576:1476
PK_N = 1476

INPUT_SPECS = [
    # branch-critical first (DMA order follows list order)
    ("packb", [128, PB_N], "bf16"),
    ("xin", [2, 128, R_XIN, 52], "bf16"),
    ("packf", [128, 8], "f32"),
    ("packk", [Cc, PK_N], "bf16"),
    ("sidx4", [96, 48], "i16"),
    ("c1", [CK, 5, 2, 3, 24], "bf16"),
    ("xt4", [96, 12, 2, 2, 128], "bf16"),
    ("projt", [128, 2, 2, 128], "bf16"),
]


def device_kernel(tc, outs, ins):
    """Emit the kernel. outs/ins: dicts name -> bass.AP (DRAM)."""
    from contextlib import ExitStack
    import concourse.bass as bass
    from concourse import mybir

    f32 = mybir.dt.float32
    bf16 = mybir.dt.bfloat16
    i16 = mybir.dt.int16
    AF = mybir.ActivationFunctionType
    ALU = mybir.AluOpType
    nc = tc.nc
    d_out = outs["out"]

    with ExitStack() as ctx:
        sing = ctx.enter_context(tc.tile_pool(name="sing", bufs=1))

        # ---- load inputs/constants (critical-path first, bulky ones later)
        sb = {}
        dts = {"bf16": bf16, "f32": f32, "i16": i16}
        deferred = ("c1", "xt4", "projt")
        for name, shape, dt in INPUT_SPECS:
            t = sing.tile(shape[:] if name != "xin" else [128, R_XIN, 52],
                          dts[dt], tag=name)
            if name == "xin":
                sb["xin0"] = t
                t2 = sing.tile([128, R_XIN, 52], bf16, tag="xin1")
                sb["xin1"] = t2
                nc.sync.dma_start(t[:], ins["xin"][0])
                nc.sync.dma_start(t2[:], ins["xin"][1])
            else:
                sb[name] = t
                if name not in deferred:
                    nc.sync.dma_start(t[:], ins[name])

        # typed views into the packs
        pb, pf = sb["packb"], sb["packf"]
        pk = sb["packk"]
        sb["w1t"] = pb[:, PB_W1T:PB_W1T + 128].rearrange(
            "p (c k) -> p c k", c=2)
        sb["ke1t"] = pk[0:Cc, PK_KE1:PK_KE1 + 576].rearrange(
            "p (t k) -> p t k", t=9)
        sb["ke2t"] = pk[0:Cc, PK_KE2:PK_KE2 + 900].rearrange(
            "p (t k) -> p t k", t=9)
        sb["gden"] = pb[0:CK, PB_GDEN:PB_GDEN + 4]
        sb["g2"] = pb[0:4, PB_G2:PB_G2 + CK]
        # only the halo rows can fall outside the image: comp rows 0:4 /
        # 16:20, k1 rows 0:2 / 14:16.  Interior rows always have mask 1.
        m1a = pb[0:Cc, PB_M1:PB_M1 + 4].unsqueeze(2).broadcast_to(
            (Cc, 4, 52))
        m1b = pb[0:Cc, PB_M1 + 16:PB_M1 + 20].unsqueeze(2).broadcast_to(
            (Cc, 4, 52))
        m2a = pb[0:Cc, PB_M2:PB_M2 + 2].unsqueeze(2).broadcast_to(
            (Cc, 2, 50))
        m2b = pb[0:Cc, PB_M2 + 14:PB_M2 + 16].unsqueeze(2).broadcast_to(
            (Cc, 2, 50))
        sb["bias1"] = pf[0:Cc, 0:1]
        sb["biask"] = pf[0:Cc, 1:2]
        bias2 = [pf[:, 2:3], pf[:, 3:4]]
        sb["etop"] = pf[0:CK, 4:6]
        sb["ebot"] = pf[0:CK, 6:8]

        # persistent activations
        comp = sing.tile([Cc, R_XIN, 52], bf16)
        k1 = sing.tile([Cc, 16, 50], bf16)
        exps = sing.tile([CK, 672], bf16)
        recs = sing.tile([4, 672], bf16)
        kn = sing.tile([CK, R_KN, 54], bf16)
        nc.vector.memset(comp[:], 0.0)
        nc.vector.memset(k1[:], 0.0)
        nc.vector.memset(kn[:], 0.0)

        # ---- branch phase
        with tc.tile_pool(name="brp", bufs=2, space="PSUM") as brp:
            for h in range(2):
                ps = brp.tile([Cc, 10 * 48], f32, tag="psc")
                for c in range(2):
                    nc.tensor.matmul(
                        ps[:], sb["w1t"][:, c, :],
                        sb[f"xin{c}"][:, h * 10:(h + 1) * 10, 2:50],
                        start=(c == 0), stop=(c == 1))
                nc.scalar.activation(
                    comp[:, h * 10:(h + 1) * 10, 2:50],
                    ps[:].rearrange("c (r w) -> c r w", w=48),
                    AF.Relu, bias=sb["bias1"][:])
                if h == 0:
                    nc.vector.tensor_mul(comp[:, 0:4, :], comp[:, 0:4, :],
                                         m1a)
                else:
                    nc.vector.tensor_mul(comp[:, 16:20, :],
                                         comp[:, 16:20, :], m1b)
            for name in deferred:
                nc.sync.dma_start(sb[name][:], ins[name])
            for h in range(2):
                ps = brp.tile([Cc, 8 * 48], f32, tag="psk1")
                for ky in range(3):
                    for kx in range(3):
                        t = ky * 3 + kx
                        nc.tensor.matmul(
                            ps[:], sb["ke1t"][:, t, :],
                            comp[:, h * 8 + ky * 2: h * 8 + ky * 2 + 8,
                                 kx * 2: kx * 2 + 48],
                            start=(t == 0), stop=(t == 8))
                nc.scalar.activation(
                    k1[:, h * 8:(h + 1) * 8, 1:49],
                    ps[:].rearrange("c (r w) -> c r w", w=48),
                    AF.Relu, bias=sb["biask"][:])
                if h == 0:
                    nc.vector.tensor_mul(k1[:, 0:2, :], k1[:, 0:2, :], m2a)
                else:
                    nc.vector.tensor_mul(k1[:, 14:16, :], k1[:, 14:16, :],
                                         m2b)
            for h in range(2):
                ps = brp.tile([CK, 7 * 48], f32, tag="psk2", bufs=2)
                for ky in range(3):
                    for kx in range(3):
                        t = ky * 3 + kx
                        nc.tensor.matmul(
                            ps[:], sb["ke2t"][:, t, :],
                            k1[:, h * 7 + ky: h * 7 + ky + 7, kx: kx + 48],
                            start=(t == 0), stop=(t == 8))
                nc.scalar.activation(exps[:, h * 336:(h + 1) * 336], ps[:],
                                     AF.Exp)
            for h in range(2):
                psd = brp.tile([4, 336], f32, tag="psc")
                nc.tensor.matmul(psd[:], sb["gden"][:],
                                 exps[:, h * 336:(h + 1) * 336],
                                 start=True, stop=True)
                with nc.allow_low_precision(reason="bf16 softmax pipeline"):
                    nc.vector.reciprocal(recs[:, h * 336:(h + 1) * 336],
                                         psd[:])
            for h in range(2):
                psb = brp.tile([CK, 336], f32, tag="psc")
                nc.tensor.matmul(psb[:], sb["g2"][:],
                                 recs[:, h * 336:(h + 1) * 336],
                                 start=True, stop=True)
                nc.vector.tensor_mul(
                    kn[:, h * 7:(h + 1) * 7, 3:51],
                    exps[:, h * 336:(h + 1) * 336].rearrange(
                        "c (r w) -> c r w", w=48),
                    psb[:].rearrange("c (r w) -> c r w", w=48))
                rs = slice(h * 7, (h + 1) * 7)
                nc.vector.tensor_copy(kn[:, rs, 2:3], kn[:, rs, 3:4])
                nc.vector.tensor_copy(kn[:, rs, 51:52], kn[:, rs, 50:51])
                if h == 0:
                    nc.vector.tensor_scalar_mul(kn[:, 0, :], kn[:, 0, :],
                                                sb["etop"][:, 1:2])
                    nc.vector.scalar_tensor_tensor(
                        kn[:, 0, :], kn[:, 1, :], sb["etop"][:, 0:1],
                        kn[:, 0, :], op0=ALU.mult, op1=ALU.add)
                else:
                    nc.vector.tensor_scalar_mul(kn[:, 13, :], kn[:, 13, :],
                                                sb["ebot"][:, 1:2])
                    nc.vector.scalar_tensor_tensor(
                        kn[:, 13, :], kn[:, 12, :], sb["ebot"][:, 0:1],
                        kn[:, 13, :], op0=ALU.mult, op1=ALU.add)

        # ---- main loop
        kbpool = ctx.enter_context(tc.tile_pool(name="kb", bufs=3))
        dpool = ctx.enter_context(tc.tile_pool(name="dsb", bufs=4))
        bpool = ctx.enter_context(tc.tile_pool(name="bb", bufs=6))
        epool = ctx.enter_context(tc.tile_pool(name="esb", bufs=2))
        e2pool = ctx.enter_context(tc.tile_pool(name="e2", bufs=2))
        spool = ctx.enter_context(tc.tile_pool(name="stage", bufs=4))
        pp_t = ctx.enter_context(tc.tile_pool(name="pp_t", bufs=2, space="PSUM"))
        pp_e = ctx.enter_context(tc.tile_pool(name="pp_e", bufs=3, space="PSUM"))
        pp_f = ctx.enter_context(tc.tile_pool(name="pp_f", bufs=3, space="PSUM"))

        esb = {}
        e2s = {}
        stage = {}
        bts = {}

        def emit_tdb(j):
            # row-blend + fine-band T collapse + band scatter for row j
            kb = kbpool.tile([CK, 2, 54], bf16, tag="kb", name=f"kb{j}")
            nc.vector.scalar_tensor_tensor(
                kb[:], kn[:, j:j + 3:2, :], 1.0 / 3.0,
                kn[:, j + 1:j + 2, :].broadcast_to((CK, 2, 54)),
                op0=ALU.mult, op1=ALU.add)
            for half in range(2):
                pst = pp_t.tile([96, 48], f32, tag="t",
                                name=f"pst{j}_{half}")
                for dy_i in range(3):
                    for p in range(2):
                        for si in range(5):
                            w0 = half * 24 + si
                            nc.tensor.matmul(
                                pst[dy_i * 32: dy_i * 32 + 26,
                                    p * 24:(p + 1) * 24],
                                kb[:, p, w0:w0 + 26],
                                sb["c1"][:, si, p, dy_i, :],
                                start=(si == 0), stop=(si == 4))
                dsb = dpool.tile([96, 48], bf16, tag="d",
                                 name=f"dsb{j}_{half}")
                nc.scalar.copy(dsb[:], pst[:])
                b4 = bpool.tile([96, 384], bf16, tag="b",
                                name=f"b4_{j}_{half}")
                nc.gpsimd.local_scatter(
                    b4[:], dsb[:], sb["sidx4"][:], channels=96,
                    num_elems=384, num_idxs=48)
                bts[(j, half)] = b4

        def emit_proj(apair):
            # proj for a completed pair: E2 is the u-folded copy of esbA,
            # so the 256-contraction takes 2 matmuls of 128 partitions.
            e2 = e2s.pop(apair)
            stg = {m: spool.tile([128, 8, 192], bf16,
                                 tag=f"s{m}", name=f"stage{apair}_{m}")
                   for m in range(2)}
            for m in range(2):
                for r in range(4):
                    u1, u2 = r // 2, r % 2
                    psf = pp_f.tile([128, 2 * 192], f32, tag="f")
                    for up in range(2):
                        nc.tensor.matmul(
                            psf[:], sb["projt"][:, up, m, :],
                            e2[:, r % 2, r // 2, up],
                            start=(up == 0), stop=(up == 1))
                    # psf cols = (a2, p, q, b); out row Y = 4*a2+2p+u1,
                    # col X = u2 + 2q + 4b
                    src = psf[:].rearrange(
                        "c (a p q b) -> c a p q b", a=2, p=2, q=2)
                    dst = stg[m][:].rearrange(
                        "c (a p v) (b q w) -> c a p v q b w",
                        a=2, p=2, b=48, q=2)[:, :, :, u1, :, :, u2]
                    if r % 2 == 0:
                        nc.scalar.activation(dst, src, AF.Relu,
                                             bias=bias2[m])
                    else:
                        nc.vector.tensor_scalar(
                            dst, src, bias2[m], 0.0,
                            op0=ALU.add, op1=ALU.max)
                dsta = bass.AP(d_out[m].tensor,
                               d_out[m].offset + apair * 1536,
                               [d_out[m].ap[0], [1, 1536]])
                nc.sync.dma_start(
                    dsta, stg[m][:].rearrange("c y x -> c (y x)"))

        emit_tdb(0)
        emit_tdb(1)
        for j in range(BLK):
            if j + 2 < BLK:
                emit_tdb(j + 2)
            if j % 2 == 0:
                esbA = epool.tile([128, 2, 4, 2, 2, 2, 48], bf16,
                                  tag="e", name=f"esb{j // 2}")
            b4h = [bts.pop((j, 0)), bts.pop((j, 1))]
            for p in range(2):
                for c in range(2):
                    pse = pp_e.tile([128, 384], f32, tag="e")
                    for half in range(2):
                        nc.tensor.matmul(
                            pse[:, half * 192:(half + 1) * 192],
                            sb["xt4"][0:90, j, half, c, :],
                            b4h[half][0:90, p * 192:(p + 1) * 192],
                            start=True, stop=True)
                    for half in range(2):
                        src = pse[:, half * 192:(half + 1) * 192].rearrange(
                            "c (u q b) -> c u q b", u=4, q=2)
                        dst = esbA[:, c, :, j % 2, p,
                                   :, half * 24:half * 24 + 24]
                        if (p + c + half) % 2 == 0:
                            nc.vector.tensor_copy(dst, src)
                        else:
                            nc.scalar.copy(dst, src)

            if j % 2 == 1:
                apair = j // 2
                # u-fold shuffle: E2[par*64+mm, (h, c, up, jpqb)] =
                #   esbA[h*64+mm, (c, u=2*up+par, jpqb)]; r = 2c+h
                e2 = e2pool.tile([128, 2, 2, 2, 2, 2, 2, 48], bf16,
                                 tag="e2", name=f"e2_{apair}")
                for par in range(2):
                    for h in range(2):
                        srcv = esbA[h * 64:(h + 1) * 64, :, par::2].rearrange(
                            "m c u j p q b -> m (c u) (j p q b)")
                        dstv = e2[par * 64:(par + 1) * 64, h].rearrange(
                            "m c u j p q b -> m (c u) (j p q b)")
                        nc.sync.dma_start(dstv, srcv)
                e2s[apair] = e2
                if apair >= 1:
                    emit_proj(apair - 1)
        emit_proj(5)


def _build_program():
    import concourse.bacc as bacc
    import concourse.tile as tile
    from concourse import mybir

    dts = {"bf16": mybir.dt.bfloat16, "f32": mybir.dt.float32,
           "i16": mybir.dt.int16}
    nc = bacc.Bacc("TRN2", target_bir_lowering=False, debug=False,
                   num_devices=NCORES)
    ins = {}
    for name, shape, dt in INPUT_SPECS:
        ins[name] = nc.dram_tensor(name, shape, dts[dt],
                                   kind="ExternalInput").ap()
    outs = {"out": nc.dram_tensor("out", [2, 128, 48 * FW],
                                  mybir.dt.bfloat16,
                                  kind="ExternalOutput").ap()}
    with tile.TileContext(nc) as tc:
        device_kernel(tc, outs, ins)
    nc.compile()
    return nc


def _get_program():
    if "nc" not in _CACHE:
        _CACHE["nc"] = _build_program()
    return _CACHE["nc"]


def _core_in_map(hp, core):
    import ml_dtypes
    bf16 = ml_dtypes.bfloat16
    cs = core_slices(hp, core)

    w1t = np.stack([hp["W1"][:, :128].T, hp["W1"][:, 128:].T],
                   axis=1)                            # [128, 2, 64]
    ke1t = np.zeros((Cc, 9, Cc), dtype=np.float32)
    ke2t = np.zeros((Cc, 9, CK), dtype=np.float32)
    for ky in range(3):
        for kx in range(3):
            t = ky * 3 + kx
            ke1t[:, t, :] = hp["W2"][:, :, ky, kx].T
            ke2t[:, t, :] = hp["W3"][:, :, ky, kx].T
    projt = np.zeros((128, 2, 2, 128), dtype=np.float32)
    for u in range(4):
        par, up = u % 2, u // 2
        for m in range(2):
            projt[par * 64:(par + 1) * 64, up, m, :] = \
                hp["P"][m * 128:(m + 1) * 128, u * 64:(u + 1) * 64].T
    gden = np.zeros((CK, 4), dtype=np.float32)
    for u in range(4):
        gden[u * 25:(u + 1) * 25, u] = 1.0
    g2 = np.ascontiguousarray(gden.T)

    packb = np.zeros((128, PB_N), dtype=np.float32)
    packb[:, PB_W1T:PB_W1T + 128] = w1t.reshape(128, 128)
    packb[0:CK, PB_GDEN:PB_GDEN + 4] = gden
    packb[0:4, PB_G2:PB_G2 + CK] = g2
    packb[0:Cc, PB_M1:PB_M1 + R_XIN] = cs["mask_comp"][None, :]
    packb[0:Cc, PB_M2:PB_M2 + 16] = cs["mask_k1"][None, :]

    packk = np.zeros((Cc, PK_N), dtype=np.float32)
    packk[:, PK_KE1:PK_KE1 + 576] = ke1t.reshape(Cc, 576)
    packk[:, PK_KE2:PK_KE2 + 900] = ke2t.reshape(Cc, 900)

    packf = np.zeros((128, 8), dtype=np.float32)
    packf[0:Cc, 0] = hp["bias1"]
    packf[0:Cc, 1] = hp["biask"]
    packf[:, 2] = hp["bias2"][:128]
    packf[:, 3] = hp["bias2"][128:]
    packf[0:CK, 4] = cs["e_top"]
    packf[0:CK, 5] = 1.0 - cs["e_top"]
    packf[0:CK, 6] = cs["e_bot"]
    packf[0:CK, 7] = 1.0 - cs["e_bot"]

    return {
        "packf": packf,
        "packb": packb.astype(bf16),
        "packk": packk.astype(bf16),
        "xin": cs["xin"].astype(bf16),
        "sidx4": hp["sidx4"],
        "c1": hp["c1"].astype(bf16),
        "xt4": cs["xt4"].astype(bf16),
        "projt": projt.astype(bf16),
    }


def _gather(results):
    out = np.zeros((N, C, FH, FW), dtype=np.float32)
    for core in range(NCORES):
        n, blk = core // 4, core % 4
        a0 = blk * BLK
        o = np.asarray(results[core]["out"]).astype(np.float32)
        o = o.reshape(2, 128, 48, FW)
        out[n, :128, 4 * a0: 4 * a0 + 48, :] = o[0]
        out[n, 128:, 4 * a0: 4 * a0 + 48, :] = o[1]
    return out


def kernel(**inputs) -> np.ndarray:
    if os.environ.get("CARAFE_EMULATE"):
        return emulate(inputs)
    from concourse.bass_utils import run_bass_kernel_spmd
    hp = host_prep(inputs)
    nc = _get_program()
    in_maps = [_core_in_map(hp, core) for core in range(NCORES)]
    res = run_bass_kernel_spmd(nc, in_maps, list(range(NCORES)),
                               trace=bool(os.environ.get("CARAFE_TRACE")))
    _CACHE["last_results"] = res
    return _gather(res.results)

